# revision 20
# baseline (speedup 1.0000x reference)
"""Trainium2 Bass kernel for a dense transformer block (B=4, N=2048, C=768,
H=12, D=64, HID=3072), sharded over 8 NeuronCores.

Sharding: token-split, no collectives. Core s handles batch b = s//2,
sequence half = s%2 (1024 tokens). Each core receives its batch element's
full 2048-token x (rolled so its own tokens are rows 0..1023), computes
K/V over all 2048 tokens (redundantly with its pair core), and produces
the output for its own 1024 tokens. Host gathers/transposes.

Layout: activations are kept feature-major ("X^T", [C, tokens]) so every
linear layer is a natural PE matmul (weights pre-transposed on host).
Attention computes S^T = K^T-tiles.T @ Q^T per head with softmax along
the partition (key) axis; the two heads of a feature-tile pair run
concurrently on the PE via tile_position row packing and share ONE
[128,1024] 2-bank PSUM tile so a single paired Exp activation covers
both. Denominators come from a ones-column appended to V; normalization
via GPSIMD partition-broadcast.

Pipelining: the kernel is emission-order software-pipelined so the
ScalarE exp stream (the attention bottleneck) always has PE work
running underneath it, keeping the PE HAM clock warm:
  D: attention(chunk0) interleaved with K/Q generation for the next
     head pair.
  F: attention(chunk1) interleaved with the first MLP quarter
     (fc1+fc2 of hidden-half 0, chunk 0).
  H: remaining three MLP quarters back-to-back (PE dense).
QKV/attention/proj and the MLP run in bf16 (fp32 PSUM accumulation).
"""

import numpy as np

import concourse.bass as bass
import concourse.mybir as mybir
import concourse.tile as tile
from concourse import bacc
from concourse.bass_utils import run_bass_kernel_spmd
from concourse.masks import make_identity

F32 = mybir.dt.float32
BF16 = mybir.dt.bfloat16
AF = mybir.ActivationFunctionType
ALU = mybir.AluOpType

B, N, C = 4, 2048, 768
H, D = 12, 64
HID = 3072
EPS = 1e-5
NCORES = 8
NO = 1024  # tokens owned per core
NKV = 2048  # key/value tokens per core
CT = C // 128  # 6 feature tiles
HT = HID // 128  # 24 hidden tiles
HQ = HT // 2  # 12 hidden tiles per half
KT = NKV // 128  # 16 kv token tiles
QCH = NO // 512  # 2 query chunks of 512
ISCALE = 1.0 / np.sqrt(D)

LAST_RESULTS = None
_NC_CACHE = None


def build_program(repeats=1):
    nc = bacc.Bacc(trn_type="TRN2", target_bir_lowering=False, num_devices=NCORES)

    xb = nc.dram_tensor("xb", [NKV, C], F32, kind="ExternalInput").ap()
    wqkvT = nc.dram_tensor("wqkvT", [C, 3 * C], BF16, kind="ExternalInput").ap()
    wprojT = nc.dram_tensor("wprojT", [C, C], BF16, kind="ExternalInput").ap()
    wfc1T = nc.dram_tensor("wfc1T", [C, HID], BF16, kind="ExternalInput").ap()
    wfc2T = nc.dram_tensor("wfc2T", [HID, C], BF16, kind="ExternalInput").ap()
    pb = nc.dram_tensor("pb", [C], F32, kind="ExternalInput").ap()
    f1b = nc.dram_tensor("f1b", [HID], F32, kind="ExternalInput").ap()
    f2b = nc.dram_tensor("f2b", [C], F32, kind="ExternalInput").ap()
    g1 = nc.dram_tensor("g1", [C], F32, kind="ExternalInput").ap()
    b1 = nc.dram_tensor("b1", [C], F32, kind="ExternalInput").ap()
    g2 = nc.dram_tensor("g2", [C], F32, kind="ExternalInput").ap()
    b2 = nc.dram_tensor("b2", [C], F32, kind="ExternalInput").ap()
    outT = nc.dram_tensor("outT", [C, NO], F32, kind="ExternalOutput").ap()

    with tile.TileContext(nc) as tc:
        for _ in range(repeats):
            emit(nc, tc, xb, wqkvT, wprojT, wfc1T, wfc2T, pb, f1b, f2b,
                 g1, b1, g2, b2, outT)
    nc.compile()
    return nc


def emit(nc, tc, xb, wqkvT, wprojT, wfc1T, wfc2T, pb, f1b, f2b,
         g1, b1, g2, b2, outT):
    dma = nc.sync.dma_start
    from contextlib import ExitStack

    with ExitStack() as top:
        lpool = lambda name, bufs: top.enter_context(
            tc.tile_pool(name=name, bufs=bufs))
        rpool = lambda name, bufs: top.enter_context(
            tc.tile_pool(name=name, bufs=bufs, side="right"))
        # ---- left side: constants + attention-phase working set ----
        consts = lpool("consts", 1)
        x2T_pool = lpool("x2T", 1)
        oT_pool = lpool("oT", 1)
        p_sb = lpool("p_sb", 3)
        asm = lpool("attn_sm", 2)
        wsb = lpool("work_sb", 2)
        wp_pool = lpool("wproj", 1)
        # ---- right side: MLP-persistent pools (live into phase H) ----
        g_pool = rpool("g_sb", 1)
        h2_pool = rpool("h2c", 1)
        acc_pool = rpool("acc_sb", 1)
        osb = rpool("out_sb", 2)
        bc_pool = rpool("bc_sb", 1)
        l2s = rpool("ln2_sm", 1)

        # ---- constants ----
        ident = consts.tile([128, 128], F32, tag="ident")
        make_identity(nc, ident)
        ident_bf = consts.tile([128, 128], BF16, tag="ident_bf")
        make_identity(nc, ident_bf)
        ones_f32 = consts.tile([128, 1], F32, tag="ones_f32")
        nc.vector.memset(ones_f32, 1.0)
        ones_cb = consts.tile([128, 1], BF16, tag="ones_cb")
        nc.scalar.activation(out=ones_cb, in_=ones_f32, func=AF.Copy)
        eps_t = consts.tile([128, 1], F32, tag="eps")
        nc.vector.memset(eps_t, EPS)
        # per-feature vectors as [128, CT] (col ct = features ct*128..)
        g1_s = consts.tile([128, CT], F32, tag="g1")
        dma(out=g1_s, in_=g1.rearrange("(ct p) -> p ct", p=128))
        b1_s = consts.tile([128, CT], F32, tag="b1")
        dma(out=b1_s, in_=b1.rearrange("(ct p) -> p ct", p=128))
        g2_s = consts.tile([128, CT], F32, tag="g2")
        dma(out=g2_s, in_=g2.rearrange("(ct p) -> p ct", p=128))
        b2_s = consts.tile([128, CT], F32, tag="b2")
        dma(out=b2_s, in_=b2.rearrange("(ct p) -> p ct", p=128))
        pb_s = consts.tile([128, CT], F32, tag="pb")
        dma(out=pb_s, in_=pb.rearrange("(ct p) -> p ct", p=128))
        f2b_s = consts.tile([128, CT], F32, tag="f2b")
        dma(out=f2b_s, in_=f2b.rearrange("(ct p) -> p ct", p=128))
        f1b_s = consts.tile([128, HT], F32, tag="f1b")
        dma(out=f1b_s, in_=f1b.rearrange("(ht p) -> p ht", p=128))

        x2T = [x2T_pool.tile([128, NO], BF16, tag=f"x2T{ct}", name=f"x2T{ct}")
               for ct in range(CT)]
        oT = [oT_pool.tile([128, 512], BF16, tag=f"oT{ct}", name=f"oT{ct}")
              for ct in range(CT)]
        wp = wp_pool.tile([128, CT, C], BF16, tag="wproj")
        g_sb = [g_pool.tile([128, 512], BF16, tag=f"g{ht}", name=f"g{ht}")
                for ht in range(HQ)]
        h2c = [[h2_pool.tile([128, 512], BF16, tag=f"h2_{ct}_{ch}",
                             name=f"h2_{ct}_{ch}") for ct in range(CT)]
               for ch in range(QCH)]
        acc = [[acc_pool.tile([128, 512], BF16, tag=f"acc{ft}_{ch}",
                              name=f"acc{ft}_{ch}") for ch in range(QCH)]
               for ft in range(CT)]
        ln2v = [None, None]
        qs = (slice(0, 64), slice(64, 128))
        qT, kT, vA, hkvT = [], [], [], []

        def h2c_apply(ch):
            """LN2 apply for chunk ch: h2c = (x2T*rstd + nmr)*g2 + b2 (DVE)."""
            cs = slice(ch * 512, (ch + 1) * 512)
            rstd, nmr = ln2v[ch]
            bc_r = bc_pool.tile([128, 512], F32, tag="bc_r")
            nc.gpsimd.partition_broadcast(bc_r, rstd, channels=128)
            bc_m = bc_pool.tile([128, 512], F32, tag="bc_m")
            nc.gpsimd.partition_broadcast(bc_m, nmr, channels=128)
            for ct in range(CT):
                t = osb.tile([128, 512], F32, tag="h2tmp")
                nc.vector.tensor_mul(t, x2T[ct][:, cs], bc_r)
                t2 = osb.tile([128, 512], F32, tag="h2tmp")
                nc.vector.tensor_add(t2, t, bc_m)
                nc.vector.tensor_scalar(
                    out=h2c[ch][ct], in0=t2,
                    scalar1=g2_s[:, ct:ct + 1], op0=ALU.mult,
                    scalar2=b2_s[:, ct:ct + 1], op1=ALU.add)

        def make_fc1(pool, w1t, hb, ch, ht):
            def f():
                ps = pool.tile([128, 512], F32, tag=pool._ktag)
                for ct in range(CT):
                    nc.tensor.matmul(
                        ps, w1t[:, ct, ht * 128:(ht + 1) * 128], h2c[ch][ct],
                        start=(ct == 0), stop=(ct == CT - 1))
                hti = hb * HQ + ht
                nc.scalar.activation(out=g_sb[ht], in_=ps, func=AF.Gelu,
                                     bias=f1b_s[:, hti:hti + 1], scale=1.0)
            return f

        def make_fc2(pool, w2t, hb, ch, ft):
            def f():
                cs = slice(ch * 512, (ch + 1) * 512)
                ps = pool.tile([128, 512], F32, tag=pool._ktag)
                for ht in range(HQ):
                    nc.tensor.matmul(
                        ps, w2t[:, ht, ft * 128:(ft + 1) * 128], g_sb[ht],
                        start=(ht == 0), stop=(ht == HQ - 1))
                if hb == 0:
                    nc.vector.scalar_tensor_tensor(
                        out=acc[ft][ch], in0=ps,
                        scalar=f2b_s[:, ft:ft + 1], in1=x2T[ft][:, cs],
                        op0=ALU.add, op1=ALU.add)
                else:
                    ot = osb.tile([128, 512], F32, tag="ot")
                    nc.vector.tensor_add(ot, ps, acc[ft][ch])
                    dma(out=outT[ft * 128:(ft + 1) * 128, cs], in_=ot)
            return f

        def proj_ln2(ch, pool, tag):
            """proj + residual into x2T (in place), then LN2 stats."""
            cs = slice(ch * 512, (ch + 1) * 512)
            for ft in range(CT):
                ps = pool.tile([128, 512], F32, tag=tag, name="ppj")
                for ct in range(CT):
                    nc.tensor.matmul(
                        ps, wp[:, ct, ft * 128:(ft + 1) * 128], oT[ct],
                        start=(ct == 0), stop=(ct == CT - 1))
                nc.vector.scalar_tensor_tensor(
                    out=x2T[ft][:, cs], in0=ps, scalar=pb_s[:, ft:ft + 1],
                    in1=x2T[ft][:, cs], op0=ALU.add, op1=ALU.add)
            psum = pool.tile([1, 512], F32, tag=tag, name="psum")
            pssq = pool.tile([1, 512], F32, tag=tag, name="pssq")
            for ct in range(CT):
                sq = wsb.tile([128, 512], BF16, tag="sq", bufs=1)
                nc.vector.tensor_mul(sq, x2T[ct][:, cs], x2T[ct][:, cs])
                nc.tensor.matmul(psum, ones_cb, x2T[ct][:, cs],
                                 start=(ct == 0), stop=(ct == CT - 1),
                                 skip_group_check=True)
                nc.tensor.matmul(pssq, ones_cb, sq,
                                 start=(ct == 0), stop=(ct == CT - 1),
                                 skip_group_check=True)
            mu = l2s.tile([1, 512], F32, tag="mu")
            nc.scalar.mul(mu, psum, 1.0 / C)
            msq = l2s.tile([1, 512], F32, tag="msq")
            nc.scalar.mul(msq, pssq, 1.0 / C)
            mu2 = l2s.tile([1, 512], F32, tag="tmpa")
            nc.vector.tensor_mul(mu2, mu, mu)
            # var in place over msq, then sd in place over mu2
            nc.vector.tensor_sub(msq, msq, mu2)
            nc.scalar.activation(out=mu2, in_=msq, func=AF.Sqrt,
                                 bias=eps_t[0:1], scale=1.0)
            rstd = l2s.tile([1, 512], F32, tag=f"rstd2_{ch}", name="rstd2")
            nc.vector.reciprocal(out=rstd, in_=mu2)
            nmr = l2s.tile([1, 512], F32, tag=f"nmr2_{ch}", name="nmr2")
            nc.vector.scalar_tensor_tensor(
                out=nmr, in0=mu, scalar=-1.0, in1=rstd,
                op0=ALU.mult, op1=ALU.mult)
            ln2v[ch] = (rstd, nmr)

        with (
            tc.tile_pool(name="sp_psum", bufs=2, space="PSUM") as sps,
            tc.tile_pool(name="o_psum", bufs=2, space="PSUM") as ops,
        ):
            def attn_hp(hp, ch):
                """Attention for head pair hp on query chunk ch."""
                qch = slice(ch * 512, (ch + 1) * 512)

                def s_pair(nt):
                    ps = sps.tile([128, 1024], F32, tag="sp", name="spS")
                    for i in range(2):
                        nc.tensor.matmul(
                            ps[:, i * 512:(i + 1) * 512],
                            kT[hp][qs[i], nt * 128:(nt + 1) * 128],
                            qT[hp][qs[i], qch],
                            start=True, stop=True,
                            tile_position=(64 * i, 0))
                    return ps

                po = [ops.tile([D + 1, 512], F32, tag="po", name="po")
                      for _ in range(2)]
                ps_cur = s_pair(0)
                for nt in range(KT):
                    ps_next = s_pair(nt + 1) if nt < KT - 1 else None
                    pt = p_sb.tile([128, 1024], BF16, tag="pt")
                    nc.scalar.activation(out=pt, in_=ps_cur,
                                         func=AF.Exp, scale=ISCALE)
                    for i in range(2):
                        nc.tensor.matmul(
                            po[i], vA[nt][:, 2 * hp + i, :],
                            pt[:, i * 512:(i + 1) * 512],
                            start=(nt == 0), stop=(nt == KT - 1),
                            skip_group_check=True)
                    ps_cur = ps_next
                for i in range(2):
                    # evacuate po to SBUF right away: frees the PSUM bank
                    # so the next head pair's PV chain isn't gated on the
                    # recip/broadcast/mul tail below.
                    po_sb = asm.tile([D + 1, 512], F32, tag="po_sb")
                    nc.vector.tensor_copy(out=po_sb, in_=po[i])
                    rec = asm.tile([1, 512], BF16, tag="rec")
                    with nc.allow_low_precision(
                            reason="softmax denom in bf16"):
                        nc.vector.reciprocal(out=rec, in_=po_sb[D:D + 1, :])
                    vb = asm.tile([D, 512], BF16, tag="vb")
                    nc.gpsimd.partition_broadcast(vb, rec, channels=D)
                    nc.vector.tensor_mul(
                        oT[hp][qs[i], :], po_sb[0:D, :], vb)

            def kq_unit(ft):
                for chh in range(NKV // 512):
                    ps = emit.mmq.tile([128, 512], F32, tag="mmq")
                    for ct in range(CT):
                        nc.tensor.matmul(
                            ps, wqk[:, ct, C + ft * 128:C + (ft + 1) * 128],
                            hkvT[ct][:, chh * 512:(chh + 1) * 512],
                            start=(ct == 0), stop=(ct == CT - 1))
                    nc.vector.tensor_copy(
                        out=kT[ft][:, chh * 512:(chh + 1) * 512], in_=ps)
                for chh in range(QCH):
                    ps = emit.mmq.tile([128, 512], F32, tag="mmq")
                    for ct in range(CT):
                        nc.tensor.matmul(
                            ps, wqk[:, ct, ft * 128:(ft + 1) * 128],
                            hkvT[ct][:, chh * 512:(chh + 1) * 512],
                            start=(ct == 0), stop=(ct == CT - 1))
                    nc.vector.tensor_copy(
                        out=qT[ft][:, chh * 512:(chh + 1) * 512], in_=ps)

            # ============ Phase A: LN1, transposes ============
            kv_stack = ExitStack()
            hkvT_pool = kv_stack.enter_context(
                tc.tile_pool(name="hkvT", bufs=1, side="right"))
            wqk_pool = kv_stack.enter_context(
                tc.tile_pool(name="wqkv_kq", bufs=1, side="right"))
            wv_stack = ExitStack()
            wqv_pool = wv_stack.enter_context(
                tc.tile_pool(name="wqkv_v", bufs=1, side="right"))
            hkvT.extend(hkvT_pool.tile([128, NKV], BF16, tag=f"hkvT{ct}",
                                       name=f"hkvT{ct}") for ct in range(CT))
            wqk = wqk_pool.tile([128, CT, 2 * C], BF16, tag="wqkv_kq")
            wqv = wqv_pool.tile([128, CT, C], BF16, tag="wqkv_v")
            with (
                tc.tile_pool(name="ln1_work", bufs=2) as lw,
                tc.tile_pool(name="ln1_stat", bufs=6) as lstat,
            ):
                for g in range(KT // 4):  # groups of 4 token tiles
                    xts, xcs = [], []
                    for j in range(4):
                        nt = 4 * g + j
                        xt = lw.tile([128, C], F32, tag=f"xt{j}",
                                     name=f"xt{j}")
                        dma(out=xt, in_=xb[nt * 128:(nt + 1) * 128, :])
                        st = lstat.tile([128, 3, 6], F32, tag="st")
                        xg = xt.rearrange("p (s d) -> p s d", s=3)
                        for s in range(3):
                            nc.vector.bn_stats(out=st[:, s], in_=xg[:, s])
                        mv = lstat.tile([128, 2], F32, tag="mv")
                        nc.vector.bn_aggr(out=mv, in_=st)
                        rstd = lstat.tile([128, 1], F32, tag="rstd")
                        nc.scalar.activation(out=rstd, in_=mv[:, 1:2],
                                             func=AF.Sqrt,
                                             bias=eps_t, scale=1.0)
                        nc.vector.reciprocal(out=rstd, in_=rstd)
                        nmr = lstat.tile([128, 1], F32, tag="nmr")
                        nc.vector.tensor_scalar(
                            out=nmr, in0=mv[:, 0:1], scalar1=-1.0,
                            scalar2=rstd, op0=ALU.mult, op1=ALU.mult)
                        xc = lw.tile([128, C], BF16, tag=f"xc{j}",
                                     name=f"xc{j}", bufs=1)
                        nc.scalar.activation(out=xc, in_=xt,
                                             func=AF.Identity,
                                             scale=rstd, bias=nmr)
                        xts.append(xt)
                        xcs.append(xc)
                    if g == 0:
                        # land in the DMA queue before QKV matmuls need them
                        dma(out=wqv,
                            in_=wqkvT.rearrange("(ct p) f -> p ct f",
                                                p=128)[:, :, 2 * C:3 * C])
                        dma(out=wqk,
                            in_=wqkvT.rearrange("(ct p) f -> p ct f",
                                                p=128)[:, :, 0:2 * C])
                    for ct in range(CT):
                        ps = sps.tile([128, 1024], BF16, tag="sp",
                                      name="ptr")
                        for j in range(4):
                            nc.tensor.transpose(
                                ps[:, j * 128:(j + 1) * 128],
                                xcs[j][:, ct * 128:(ct + 1) * 128],
                                ident_bf)
                        nc.scalar.activation(
                            out=hkvT[ct][:, g * 512:(g + 1) * 512],
                            in_=ps[:, 0:512], func=AF.Identity,
                            scale=g1_s[:, ct:ct + 1], bias=b1_s[:, ct:ct + 1])
                    if g < NO // 512:  # own tokens: raw x^T for residual
                        for ct in range(CT):
                            ps32 = sps.tile([128, 1024], F32, tag="sp",
                                            name="ptr32")
                            for j in range(4):
                                nc.tensor.transpose(
                                    ps32[:, j * 128:(j + 1) * 128],
                                    xts[j][:, ct * 128:(ct + 1) * 128],
                                    ident)
                            nc.vector.tensor_copy(
                                out=x2T[ct][:, g * 512:(g + 1) * 512],
                                in_=ps32[:, 0:512])
                # proj weights: queue behind x/wq so LN1 isn't delayed
                dma(out=wp,
                    in_=wprojT.rearrange("(ct p) f -> p ct f", p=128))

            # ============ Phase B: V ============
            qkv_stack = ExitStack()
            emit.qkv_stack = qkv_stack
            vA_pool = qkv_stack.enter_context(
                tc.tile_pool(name="vA", bufs=1))
            vA.extend(vA_pool.tile([128, H, D + 1], BF16, tag=f"vA{nt}",
                                   name=f"vA{nt}") for nt in range(KT))
            for nt in range(KT):
                psv = sps.tile([128, 1024], F32, tag="sp", name="psv")
                for ct in range(CT):
                    hk = hkvT[ct][:, nt * 128:(nt + 1) * 128]
                    nc.tensor.matmul(psv[:, 0:512], hk,
                                     wqv[:, ct, 0:512],
                                     start=(ct == 0), stop=(ct == CT - 1))
                    nc.tensor.matmul(psv[:, 512:768], hk,
                                     wqv[:, ct, 512:C],
                                     start=(ct == 0), stop=(ct == CT - 1))
                nc.vector.tensor_copy(
                    out=vA[nt][:, 0:8, 0:D],
                    in_=psv[:, 0:512].rearrange("p (h d) -> p h d", d=D))
                nc.vector.tensor_copy(
                    out=vA[nt][:, 8:12, 0:D],
                    in_=psv[:, 512:768].rearrange("p (h d) -> p h d", d=D))
                nc.vector.memset(vA[nt][:, :, D:D + 1], 1.0)
            wv_stack.close()  # wqv freed

            # ===== Phases C/D/E: K/Q + attention(ch0) + proj/LN2(ch0) =====
            qT_pool = qkv_stack.enter_context(
                tc.tile_pool(name="qT", bufs=1))
            kT_pool = qkv_stack.enter_context(
                tc.tile_pool(name="kT", bufs=1))
            qT.extend(qT_pool.tile([128, NO], BF16, tag=f"qT{ct}",
                                   name=f"qT{ct}") for ct in range(CT))
            kT.extend(kT_pool.tile([128, NKV], BF16, tag=f"kT{ct}",
                                   name=f"kT{ct}") for ct in range(CT))
            with tc.tile_pool(name="mmq_psum", bufs=2, space="PSUM") as mmq:
                emit.mmq = mmq
                kq_unit(0)
                for hp in range(CT):
                    if hp + 1 < CT:
                        kq_unit(hp + 1)
                    attn_hp(hp, 0)
                proj_ln2(0, mmq, "mmq")
                h2c_apply(0)
            kv_stack.close()  # hkvT + K/Q weights freed

            # ===== Phase F: attention(ch1) + MLP quarter (hb0, ch0) =====
            with (
                tc.tile_pool(name="wfc1f", bufs=1, side="right") as w1f_pool,
                tc.tile_pool(name="wfc2f", bufs=1, side="right") as w2f_pool,
                tc.tile_pool(name="f_psum", bufs=2, space="PSUM") as fps,
            ):
                fps._ktag = "f1"
                w1f = w1f_pool.tile([128, CT, HQ * 128], BF16, tag="w1f")
                dma(out=w1f,
                    in_=wfc1T.rearrange("(ct p) f -> p ct f",
                                        p=128)[:, :, 0:HQ * 128])
                w2f = w2f_pool.tile([128, HQ, C], BF16, tag="w2f")
                dma(out=w2f,
                    in_=wfc2T.rearrange("(ht p) f -> p ht f",
                                        p=128)[:, 0:HQ, :])
                filler = [
                    [],
                    [make_fc1(fps, w1f, 0, 0, ht) for ht in range(6)],
                    [make_fc1(fps, w1f, 0, 0, ht) for ht in range(6, HQ)],
                    [make_fc2(fps, w2f, 0, 0, ft) for ft in range(2)],
                    [make_fc2(fps, w2f, 0, 0, ft) for ft in range(2, 4)],
                    [make_fc2(fps, w2f, 0, 0, ft) for ft in range(4, CT)],
                ]
                for hp in range(CT):
                    for u in filler[hp]:
                        u()
                    attn_hp(hp, 1)
                proj_ln2(1, fps, "f1")
        emit.qkv_stack.close()  # qT/kT/vA freed

        # ======= Phases G/H: LN2 apply ch1 + remaining MLP quarters =======
        with (
            tc.tile_pool(name="wfc1b", bufs=1, side="right") as w1b_pool,
            tc.tile_pool(name="wfc2b", bufs=1, side="right") as w2b_pool,
            tc.tile_pool(name="ft_psum", bufs=4, space="PSUM") as fpst,
        ):
            fpst._ktag = "ft"
            w1 = {}
            w2 = {}
            w1[1] = w1b_pool.tile([128, CT, HQ * 128], BF16, tag="w1h1",
                                  name="w1h1")
            dma(out=w1[1],
                in_=wfc1T.rearrange("(ct p) f -> p ct f",
                                    p=128)[:, :, HQ * 128:HID])
            w2[1] = w2b_pool.tile([128, HQ, C], BF16, tag="w2h1",
                                  name="w2h1")
            dma(out=w2[1],
                in_=wfc2T.rearrange("(ht p) f -> p ht f", p=128)[:, HQ:HT, :])
            w1[0] = w1b_pool.tile([128, CT, HQ * 128], BF16, tag="w1h0r",
                                  name="w1h0r")
            dma(out=w1[0],
                in_=wfc1T.rearrange("(ct p) f -> p ct f",
                                    p=128)[:, :, 0:HQ * 128])
            w2[0] = w2b_pool.tile([128, HQ, C], BF16, tag="w2h0r",
                                  name="w2h0r")
            dma(out=w2[0],
                in_=wfc2T.rearrange("(ht p) f -> p ht f", p=128)[:, 0:HQ, :])
            h2c_apply(1)
            for hb, ch in ((1, 0), (0, 1), (1, 1)):
                for ht in range(HQ):
                    make_fc1(fpst, w1[hb], hb, ch, ht)()
                for ft in range(CT):
                    make_fc2(fpst, w2[hb], hb, ch, ft)()


def kernel(**inputs):
    global _NC_CACHE, LAST_RESULTS
    import os
    ins = {k: np.ascontiguousarray(np.asarray(v, dtype=np.float32))
           for k, v in inputs.items()}
    if _NC_CACHE is None:
        _NC_CACHE = build_program()
    nc = _NC_CACHE

    import ml_dtypes
    bf = ml_dtypes.bfloat16
    shared = {
        "wqkvT": np.ascontiguousarray(ins["qkv_w"].T.astype(bf)),
        "wprojT": np.ascontiguousarray(ins["proj_w"].T.astype(bf)),
        "wfc1T": np.ascontiguousarray(ins["fc1_w"].T.astype(bf)),
        "wfc2T": np.ascontiguousarray(ins["fc2_w"].T.astype(bf)),
        "pb": ins["proj_b"], "f1b": ins["fc1_b"], "f2b": ins["fc2_b"],
        "g1": ins["ln1_g"], "b1": ins["ln1_b"],
        "g2": ins["ln2_g"], "b2": ins["ln2_b"],
    }
    in_maps = []
    for s in range(NCORES):
        b, half = s // 2, s % 2
        m = dict(shared)
        m["xb"] = np.ascontiguousarray(np.roll(ins["x"][b], -half * NO, axis=0))
        in_maps.append(m)

    trace = bool(int(os.environ.get("KBENCH_TRACE", "0")))
    LAST_RESULTS = run_bass_kernel_spmd(
        nc, in_maps, core_ids=list(range(NCORES)), trace=trace)
    out = np.empty((B, N, C), np.float32)
    for s in range(NCORES):
        b, half = s // 2, s % 2
        out[b, half * NO:(half + 1) * NO, :] = LAST_RESULTS.results[s]["outT"].T
    return out


# revision 21
# speedup vs baseline: 1.1361x; 1.1361x over previous
"""Trainium2 Bass kernel for a dense transformer block (B=4, N=2048, C=768,
H=12, D=64, HID=3072), sharded over 8 NeuronCores.

Sharding: token-split, no collectives. Core s handles batch b = s//2,
sequence half = s%2 (1024 tokens). Each core receives its batch element's
full 2048-token x (rolled so its own tokens are rows 0..1023), computes
K/V over all 2048 tokens (redundantly with its pair core), and produces
the output for its own 1024 tokens. Host gathers/transposes.

Layout: activations are kept feature-major ("X^T", [C, tokens]) so every
linear layer is a natural PE matmul (weights pre-transposed on host).
Attention computes S^T = K^T-tiles.T @ Q^T per head with softmax along
the partition (key) axis; the two heads of a feature-tile pair run
concurrently on the PE via tile_position row packing and share ONE
[128,1024] 2-bank PSUM tile so a single paired Exp activation covers
both. Denominators come from a ones-column appended to V; normalization
via GPSIMD partition-broadcast.

Pipelining: the kernel is emission-order software-pipelined so the
ScalarE exp stream (the attention bottleneck) always has PE work
running underneath it, keeping the PE HAM clock warm:
  D: attention(chunk0) interleaved with K/Q generation for the next
     head pair.
  F: attention(chunk1) interleaved with the first MLP quarter
     (fc1+fc2 of hidden-half 0, chunk 0).
  H: remaining three MLP quarters back-to-back (PE dense).
QKV/attention/proj and the MLP run in bf16 (fp32 PSUM accumulation).
"""

import numpy as np

import concourse.bass as bass
import concourse.mybir as mybir
import concourse.tile as tile
from concourse import bacc
from concourse.bass_utils import run_bass_kernel_spmd
from concourse.masks import make_identity

F32 = mybir.dt.float32
BF16 = mybir.dt.bfloat16
AF = mybir.ActivationFunctionType
ALU = mybir.AluOpType

B, N, C = 4, 2048, 768
H, D = 12, 64
HID = 3072
EPS = 1e-5
NCORES = 8
NO = 1024  # tokens owned per core
NKV = 2048  # key/value tokens per core
CT = C // 128  # 6 feature tiles
HT = HID // 128  # 24 hidden tiles
HQ = HT // 2  # 12 hidden tiles per half
KT = NKV // 128  # 16 kv token tiles
QCH = NO // 512  # 2 query chunks of 512
ISCALE = 1.0 / np.sqrt(D)

LAST_RESULTS = None
_NC_CACHE = None


def build_program(repeats=1):
    nc = bacc.Bacc(trn_type="TRN2", target_bir_lowering=False, num_devices=NCORES)

    xb = nc.dram_tensor("xb", [NKV, C], F32, kind="ExternalInput").ap()
    wqkvT = nc.dram_tensor("wqkvT", [C, 3 * C], BF16, kind="ExternalInput").ap()
    wprojT = nc.dram_tensor("wprojT", [C, C], BF16, kind="ExternalInput").ap()
    wfc1T = nc.dram_tensor("wfc1T", [C, HID], BF16, kind="ExternalInput").ap()
    wfc2T = nc.dram_tensor("wfc2T", [HID, C], BF16, kind="ExternalInput").ap()
    pb = nc.dram_tensor("pb", [C], F32, kind="ExternalInput").ap()
    f1b = nc.dram_tensor("f1b", [HID], F32, kind="ExternalInput").ap()
    f2b = nc.dram_tensor("f2b", [C], F32, kind="ExternalInput").ap()
    g1 = nc.dram_tensor("g1", [C], F32, kind="ExternalInput").ap()
    b1 = nc.dram_tensor("b1", [C], F32, kind="ExternalInput").ap()
    g2 = nc.dram_tensor("g2", [C], F32, kind="ExternalInput").ap()
    b2 = nc.dram_tensor("b2", [C], F32, kind="ExternalInput").ap()
    outT = nc.dram_tensor("outT", [C, NO], F32, kind="ExternalOutput").ap()

    with tile.TileContext(nc) as tc:
        for _ in range(repeats):
            emit(nc, tc, xb, wqkvT, wprojT, wfc1T, wfc2T, pb, f1b, f2b,
                 g1, b1, g2, b2, outT)
    nc.compile()
    return nc


def emit(nc, tc, xb, wqkvT, wprojT, wfc1T, wfc2T, pb, f1b, f2b,
         g1, b1, g2, b2, outT):
    dma = nc.sync.dma_start
    from contextlib import ExitStack

    with ExitStack() as top:
        lpool = lambda name, bufs: top.enter_context(
            tc.tile_pool(name=name, bufs=bufs))
        rpool = lambda name, bufs: top.enter_context(
            tc.tile_pool(name=name, bufs=bufs, side="right"))
        # ---- left side: constants + attention-phase working set ----
        consts = lpool("consts", 1)
        x2T_pool = lpool("x2T", 1)
        oT_pool = lpool("oT", 1)
        p_sb = lpool("p_sb", 3)
        asm = lpool("attn_sm", 1)
        wsb = lpool("work_sb", 2)
        wp_pool = lpool("wproj", 1)
        # ---- right side: MLP-persistent pools (live into phase H) ----
        g_pool = rpool("g_sb", 1)
        h2_pool = rpool("h2c", 1)
        acc_pool = rpool("acc_sb", 1)
        osb = rpool("out_sb", 2)
        bc_pool = rpool("bc_sb", 1)
        l2s = rpool("ln2_sm", 1)

        # ---- constants ----
        ident = consts.tile([128, 128], F32, tag="ident")
        make_identity(nc, ident)
        ident_bf = consts.tile([128, 128], BF16, tag="ident_bf")
        make_identity(nc, ident_bf)
        ones_f32 = consts.tile([128, 1], F32, tag="ones_f32")
        nc.vector.memset(ones_f32, 1.0)
        ones_cb = consts.tile([128, 1], BF16, tag="ones_cb")
        nc.scalar.activation(out=ones_cb, in_=ones_f32, func=AF.Copy)
        eps_t = consts.tile([128, 1], F32, tag="eps")
        nc.vector.memset(eps_t, EPS)
        # per-feature vectors as [128, CT] (col ct = features ct*128..)
        g1_s = consts.tile([128, CT], F32, tag="g1")
        dma(out=g1_s, in_=g1.rearrange("(ct p) -> p ct", p=128))
        b1_s = consts.tile([128, CT], F32, tag="b1")
        dma(out=b1_s, in_=b1.rearrange("(ct p) -> p ct", p=128))
        g2_s = consts.tile([128, CT], F32, tag="g2")
        dma(out=g2_s, in_=g2.rearrange("(ct p) -> p ct", p=128))
        b2_s = consts.tile([128, CT], F32, tag="b2")
        dma(out=b2_s, in_=b2.rearrange("(ct p) -> p ct", p=128))
        pb_s = consts.tile([128, CT], F32, tag="pb")
        dma(out=pb_s, in_=pb.rearrange("(ct p) -> p ct", p=128))
        f2b_s = consts.tile([128, CT], F32, tag="f2b")
        dma(out=f2b_s, in_=f2b.rearrange("(ct p) -> p ct", p=128))
        f1b_s = consts.tile([128, HT], F32, tag="f1b")
        dma(out=f1b_s, in_=f1b.rearrange("(ht p) -> p ht", p=128))

        x2T = [x2T_pool.tile([128, NO], BF16, tag=f"x2T{ct}", name=f"x2T{ct}")
               for ct in range(CT)]
        oT = [oT_pool.tile([128, 512], BF16, tag=f"oT{ct}", name=f"oT{ct}")
              for ct in range(CT)]
        wp = wp_pool.tile([128, CT, C], BF16, tag="wproj")
        g_sb = [g_pool.tile([128, 512], BF16, tag=f"g{ht}", name=f"g{ht}")
                for ht in range(HQ)]
        h2c = [[h2_pool.tile([128, 512], BF16, tag=f"h2_{ct}_{ch}",
                             name=f"h2_{ct}_{ch}") for ct in range(CT)]
               for ch in range(QCH)]
        acc = [[acc_pool.tile([128, 512], BF16, tag=f"acc{ft}_{ch}",
                              name=f"acc{ft}_{ch}") for ch in range(QCH)]
               for ft in range(CT)]
        ln2v = [None, None]
        qs = (slice(0, 64), slice(64, 128))
        qT, kT, vA, hkvT = [], [], [], []

        def h2c_apply(ch):
            """LN2 apply for chunk ch: h2c = (x2T*rstd + nmr)*g2 + b2 (DVE)."""
            cs = slice(ch * 512, (ch + 1) * 512)
            rstd, nmr = ln2v[ch]
            bc_r = bc_pool.tile([128, 512], F32, tag="bc_r")
            nc.gpsimd.partition_broadcast(bc_r, rstd, channels=128)
            bc_m = bc_pool.tile([128, 512], F32, tag="bc_m")
            nc.gpsimd.partition_broadcast(bc_m, nmr, channels=128)
            for ct in range(CT):
                t = osb.tile([128, 512], F32, tag="h2tmp")
                nc.vector.tensor_mul(t, x2T[ct][:, cs], bc_r)
                t2 = osb.tile([128, 512], F32, tag="h2tmp")
                nc.vector.tensor_add(t2, t, bc_m)
                nc.vector.tensor_scalar(
                    out=h2c[ch][ct], in0=t2,
                    scalar1=g2_s[:, ct:ct + 1], op0=ALU.mult,
                    scalar2=b2_s[:, ct:ct + 1], op1=ALU.add)

        def make_fc1(pool, w1t, hb, ch, ht):
            def f():
                ps = pool.tile([128, 512], F32, tag=pool._ktag)
                for ct in range(CT):
                    nc.tensor.matmul(
                        ps, w1t[:, ct, ht * 128:(ht + 1) * 128], h2c[ch][ct],
                        start=(ct == 0), stop=(ct == CT - 1))
                hti = hb * HQ + ht
                nc.scalar.activation(out=g_sb[ht], in_=ps, func=AF.Gelu,
                                     bias=f1b_s[:, hti:hti + 1], scale=1.0)
            return f

        def make_fc2(pool, w2t, hb, ch, ft):
            def f():
                cs = slice(ch * 512, (ch + 1) * 512)
                ps = pool.tile([128, 512], F32, tag=pool._ktag)
                for ht in range(HQ):
                    nc.tensor.matmul(
                        ps, w2t[:, ht, ft * 128:(ft + 1) * 128], g_sb[ht],
                        start=(ht == 0), stop=(ht == HQ - 1))
                if hb == 0:
                    nc.vector.scalar_tensor_tensor(
                        out=acc[ft][ch], in0=ps,
                        scalar=f2b_s[:, ft:ft + 1], in1=x2T[ft][:, cs],
                        op0=ALU.add, op1=ALU.add)
                else:
                    ot = osb.tile([128, 512], F32, tag="ot")
                    nc.vector.tensor_add(ot, ps, acc[ft][ch])
                    dma(out=outT[ft * 128:(ft + 1) * 128, cs], in_=ot)
            return f

        def proj_ln2(ch, pool, tag):
            """proj + residual into x2T (in place), then LN2 stats."""
            cs = slice(ch * 512, (ch + 1) * 512)
            for ft in range(CT):
                ps = pool.tile([128, 512], F32, tag=tag, name="ppj")
                for ct in range(CT):
                    nc.tensor.matmul(
                        ps, wp[:, ct, ft * 128:(ft + 1) * 128], oT[ct],
                        start=(ct == 0), stop=(ct == CT - 1))
                nc.vector.scalar_tensor_tensor(
                    out=x2T[ft][:, cs], in0=ps, scalar=pb_s[:, ft:ft + 1],
                    in1=x2T[ft][:, cs], op0=ALU.add, op1=ALU.add)
            psum = pool.tile([1, 512], F32, tag=tag, name="psum")
            pssq = pool.tile([1, 512], F32, tag=tag, name="pssq")
            for ct in range(CT):
                sq = wsb.tile([128, 512], BF16, tag="sq", bufs=1)
                nc.vector.tensor_mul(sq, x2T[ct][:, cs], x2T[ct][:, cs])
                nc.tensor.matmul(psum, ones_cb, x2T[ct][:, cs],
                                 start=(ct == 0), stop=(ct == CT - 1),
                                 skip_group_check=True)
                nc.tensor.matmul(pssq, ones_cb, sq,
                                 start=(ct == 0), stop=(ct == CT - 1),
                                 skip_group_check=True)
            mu = l2s.tile([1, 512], F32, tag="mu")
            nc.scalar.mul(mu, psum, 1.0 / C)
            msq = l2s.tile([1, 512], F32, tag="msq")
            nc.scalar.mul(msq, pssq, 1.0 / C)
            mu2 = l2s.tile([1, 512], F32, tag="tmpa")
            nc.vector.tensor_mul(mu2, mu, mu)
            # var in place over msq; rstd = exp(-0.5*ln(var+eps)) keeps the
            # ACT table in the natural_log_exp set (no Sqrt-set thrash
            # between the attention exp streams).
            nc.vector.tensor_sub(msq, msq, mu2)
            nc.scalar.activation(out=mu2, in_=msq, func=AF.Ln,
                                 bias=eps_t[0:1], scale=1.0)
            rstd = l2s.tile([1, 512], F32, tag=f"rstd2_{ch}", name="rstd2")
            nc.scalar.activation(out=rstd, in_=mu2, func=AF.Exp,
                                 scale=-0.5)
            nmr = l2s.tile([1, 512], F32, tag=f"nmr2_{ch}", name="nmr2")
            nc.vector.scalar_tensor_tensor(
                out=nmr, in0=mu, scalar=-1.0, in1=rstd,
                op0=ALU.mult, op1=ALU.mult)
            ln2v[ch] = (rstd, nmr)

        with (
            tc.tile_pool(name="sp_psum", bufs=2, space="PSUM") as sps,
            tc.tile_pool(name="o_psum", bufs=2, space="PSUM") as ops,
        ):
            def attn_hp(hp, ch):
                """Attention for head pair hp on query chunk ch."""
                qch = slice(ch * 512, (ch + 1) * 512)

                def s_pair(nt):
                    ps = sps.tile([128, 1024], F32, tag="sp", name="spS")
                    for i in range(2):
                        nc.tensor.matmul(
                            ps[:, i * 512:(i + 1) * 512],
                            kT[hp][qs[i], nt * 128:(nt + 1) * 128],
                            qT[hp][qs[i], qch],
                            start=True, stop=True,
                            tile_position=(64 * i, 0))
                    return ps

                po = [ops.tile([D + 1, 512], F32, tag="po", name="po")
                      for _ in range(2)]
                ps_cur = s_pair(0)
                for nt in range(KT):
                    ps_next = s_pair(nt + 1) if nt < KT - 1 else None
                    pt = p_sb.tile([128, 1024], BF16, tag="pt")
                    nc.scalar.activation(out=pt, in_=ps_cur,
                                         func=AF.Exp, scale=ISCALE)
                    for i in range(2):
                        nc.tensor.matmul(
                            po[i], vA[nt][:, 2 * hp + i, :],
                            pt[:, i * 512:(i + 1) * 512],
                            start=(nt == 0), stop=(nt == KT - 1),
                            skip_group_check=True)
                    ps_cur = ps_next
                for i in range(2):
                    rec = asm.tile([1, 512], BF16, tag="rec")
                    with nc.allow_low_precision(
                            reason="softmax denom in bf16"):
                        nc.vector.reciprocal(out=rec, in_=po[i][D:D + 1, :])
                    vb = asm.tile([D, 512], BF16, tag="vb")
                    nc.gpsimd.partition_broadcast(vb, rec, channels=D)
                    nc.vector.tensor_mul(
                        oT[hp][qs[i], :], po[i][0:D, :], vb)

            def kq_unit(ft):
                for chh in range(NKV // 512):
                    ps = emit.mmq.tile([128, 512], F32, tag="mmq")
                    for ct in range(CT):
                        nc.tensor.matmul(
                            ps, wqk[:, ct, C + ft * 128:C + (ft + 1) * 128],
                            hkvT[ct][:, chh * 512:(chh + 1) * 512],
                            start=(ct == 0), stop=(ct == CT - 1))
                    nc.vector.tensor_copy(
                        out=kT[ft][:, chh * 512:(chh + 1) * 512], in_=ps)
                for chh in range(QCH):
                    ps = emit.mmq.tile([128, 512], F32, tag="mmq")
                    for ct in range(CT):
                        nc.tensor.matmul(
                            ps, wqk[:, ct, ft * 128:(ft + 1) * 128],
                            hkvT[ct][:, chh * 512:(chh + 1) * 512],
                            start=(ct == 0), stop=(ct == CT - 1))
                    nc.vector.tensor_copy(
                        out=qT[ft][:, chh * 512:(chh + 1) * 512], in_=ps)

            # ============ Phase A: LN1, transposes ============
            kv_stack = ExitStack()
            hkvT_pool = kv_stack.enter_context(
                tc.tile_pool(name="hkvT", bufs=1, side="right"))
            wqk_pool = kv_stack.enter_context(
                tc.tile_pool(name="wqkv_kq", bufs=1, side="right"))
            wv_stack = ExitStack()
            wqv_pool = wv_stack.enter_context(
                tc.tile_pool(name="wqkv_v", bufs=1, side="right"))
            hkvT.extend(hkvT_pool.tile([128, NKV], BF16, tag=f"hkvT{ct}",
                                       name=f"hkvT{ct}") for ct in range(CT))
            wqk = wqk_pool.tile([128, CT, 2 * C], BF16, tag="wqkv_kq")
            wqv = wqv_pool.tile([128, CT, C], BF16, tag="wqkv_v")
            with (
                tc.tile_pool(name="ln1_work", bufs=2) as lw,
                tc.tile_pool(name="ln1_stat", bufs=6) as lstat,
            ):
                for g in range(KT // 4):  # groups of 4 token tiles
                    xts, xcs = [], []
                    for j in range(4):
                        nt = 4 * g + j
                        xt = lw.tile([128, C], F32, tag=f"xt{j}",
                                     name=f"xt{j}")
                        dma(out=xt, in_=xb[nt * 128:(nt + 1) * 128, :])
                        st = lstat.tile([128, 3, 6], F32, tag="st")
                        xg = xt.rearrange("p (s d) -> p s d", s=3)
                        for s in range(3):
                            nc.vector.bn_stats(out=st[:, s], in_=xg[:, s])
                        mv = lstat.tile([128, 2], F32, tag="mv")
                        nc.vector.bn_aggr(out=mv, in_=st)
                        rstd = lstat.tile([128, 1], F32, tag="rstd")
                        nc.scalar.activation(out=rstd, in_=mv[:, 1:2],
                                             func=AF.Sqrt,
                                             bias=eps_t, scale=1.0)
                        nc.vector.reciprocal(out=rstd, in_=rstd)
                        nmr = lstat.tile([128, 1], F32, tag="nmr")
                        nc.vector.tensor_scalar(
                            out=nmr, in0=mv[:, 0:1], scalar1=-1.0,
                            scalar2=rstd, op0=ALU.mult, op1=ALU.mult)
                        xc = lw.tile([128, C], BF16, tag=f"xc{j}",
                                     name=f"xc{j}", bufs=1)
                        nc.scalar.activation(out=xc, in_=xt,
                                             func=AF.Identity,
                                             scale=rstd, bias=nmr)
                        xts.append(xt)
                        xcs.append(xc)
                    if g == 0:
                        # land in the DMA queue before QKV matmuls need them
                        nc.gpsimd.dma_start(out=wqv,
                            in_=wqkvT.rearrange("(ct p) f -> p ct f",
                                                p=128)[:, :, 2 * C:3 * C])
                        nc.gpsimd.dma_start(out=wqk,
                            in_=wqkvT.rearrange("(ct p) f -> p ct f",
                                                p=128)[:, :, 0:2 * C])
                    for ct in range(CT):
                        ps = sps.tile([128, 1024], BF16, tag="sp",
                                      name="ptr")
                        for j in range(4):
                            nc.tensor.transpose(
                                ps[:, j * 128:(j + 1) * 128],
                                xcs[j][:, ct * 128:(ct + 1) * 128],
                                ident_bf)
                        nc.scalar.activation(
                            out=hkvT[ct][:, g * 512:(g + 1) * 512],
                            in_=ps[:, 0:512], func=AF.Identity,
                            scale=g1_s[:, ct:ct + 1], bias=b1_s[:, ct:ct + 1])
                    if g < NO // 512:  # own tokens: raw x^T for residual
                        for ct in range(CT):
                            ps32 = sps.tile([128, 1024], F32, tag="sp",
                                            name="ptr32")
                            for j in range(4):
                                nc.tensor.transpose(
                                    ps32[:, j * 128:(j + 1) * 128],
                                    xts[j][:, ct * 128:(ct + 1) * 128],
                                    ident)
                            nc.vector.tensor_copy(
                                out=x2T[ct][:, g * 512:(g + 1) * 512],
                                in_=ps32[:, 0:512])
                # proj weights: queue behind x/wq so LN1 isn't delayed
                nc.gpsimd.dma_start(out=wp,
                    in_=wprojT.rearrange("(ct p) f -> p ct f", p=128))

            # ============ Phase B: V ============
            qkv_stack = ExitStack()
            emit.qkv_stack = qkv_stack
            vA_pool = qkv_stack.enter_context(
                tc.tile_pool(name="vA", bufs=1))
            vA.extend(vA_pool.tile([128, H, D + 1], BF16, tag=f"vA{nt}",
                                   name=f"vA{nt}") for nt in range(KT))
            for nt in range(KT):
                psv = sps.tile([128, 1024], F32, tag="sp", name="psv")
                for ct in range(CT):
                    hk = hkvT[ct][:, nt * 128:(nt + 1) * 128]
                    nc.tensor.matmul(psv[:, 0:512], hk,
                                     wqv[:, ct, 0:512],
                                     start=(ct == 0), stop=(ct == CT - 1))
                    nc.tensor.matmul(psv[:, 512:768], hk,
                                     wqv[:, ct, 512:C],
                                     start=(ct == 0), stop=(ct == CT - 1))
                nc.vector.tensor_copy(
                    out=vA[nt][:, 0:8, 0:D],
                    in_=psv[:, 0:512].rearrange("p (h d) -> p h d", d=D))
                nc.vector.tensor_copy(
                    out=vA[nt][:, 8:12, 0:D],
                    in_=psv[:, 512:768].rearrange("p (h d) -> p h d", d=D))
                nc.vector.memset(vA[nt][:, :, D:D + 1], 1.0)
            wv_stack.close()  # wqv freed

            # ===== Phases C/D/E: K/Q + attention(ch0) + proj/LN2(ch0) =====
            qT_pool = qkv_stack.enter_context(
                tc.tile_pool(name="qT", bufs=1))
            kT_pool = qkv_stack.enter_context(
                tc.tile_pool(name="kT", bufs=1))
            qT.extend(qT_pool.tile([128, NO], BF16, tag=f"qT{ct}",
                                   name=f"qT{ct}") for ct in range(CT))
            kT.extend(kT_pool.tile([128, NKV], BF16, tag=f"kT{ct}",
                                   name=f"kT{ct}") for ct in range(CT))
            with tc.tile_pool(name="mmq_psum", bufs=2, space="PSUM") as mmq:
                emit.mmq = mmq
                kq_unit(0)
                for hp in range(CT):
                    if hp + 1 < CT:
                        kq_unit(hp + 1)
                    attn_hp(hp, 0)
                proj_ln2(0, mmq, "mmq")
                h2c_apply(0)
            kv_stack.close()  # hkvT + K/Q weights freed

            # ===== Phase F: attention(ch1) + MLP quarter (hb0, ch0) =====
            with (
                tc.tile_pool(name="wfc1f", bufs=1, side="right") as w1f_pool,
                tc.tile_pool(name="wfc2f", bufs=1, side="right") as w2f_pool,
                tc.tile_pool(name="f_psum", bufs=2, space="PSUM") as fps,
            ):
                fps._ktag = "f1"
                w1f = w1f_pool.tile([128, CT, HQ * 128], BF16, tag="w1f")
                nc.gpsimd.dma_start(out=w1f,
                    in_=wfc1T.rearrange("(ct p) f -> p ct f",
                                        p=128)[:, :, 0:HQ * 128])
                w2f = w2f_pool.tile([128, HQ, C], BF16, tag="w2f")
                nc.gpsimd.dma_start(out=w2f,
                    in_=wfc2T.rearrange("(ht p) f -> p ht f",
                                        p=128)[:, 0:HQ, :])
                filler = [
                    [],
                    [make_fc1(fps, w1f, 0, 0, ht) for ht in range(6)],
                    [make_fc1(fps, w1f, 0, 0, ht) for ht in range(6, HQ)],
                    [make_fc2(fps, w2f, 0, 0, ft) for ft in range(3)],
                    [make_fc2(fps, w2f, 0, 0, ft) for ft in range(3, CT)],
                    [],
                ]
                for hp in range(CT):
                    for u in filler[hp]:
                        u()
                    attn_hp(hp, 1)
                proj_ln2(1, fps, "f1")
        emit.qkv_stack.close()  # qT/kT/vA freed

        # ======= Phases G/H: LN2 apply ch1 + remaining MLP quarters =======
        with (
            tc.tile_pool(name="wfc1b", bufs=1, side="right") as w1b_pool,
            tc.tile_pool(name="wfc2b", bufs=1, side="right") as w2b_pool,
            tc.tile_pool(name="ft_psum", bufs=4, space="PSUM") as fpst,
        ):
            fpst._ktag = "ft"
            w1 = {}
            w2 = {}
            w1[1] = w1b_pool.tile([128, CT, HQ * 128], BF16, tag="w1h1",
                                  name="w1h1")
            nc.gpsimd.dma_start(out=w1[1],
                in_=wfc1T.rearrange("(ct p) f -> p ct f",
                                    p=128)[:, :, HQ * 128:HID])
            w2[1] = w2b_pool.tile([128, HQ, C], BF16, tag="w2h1",
                                  name="w2h1")
            nc.gpsimd.dma_start(out=w2[1],
                in_=wfc2T.rearrange("(ht p) f -> p ht f", p=128)[:, HQ:HT, :])
            w1[0] = w1b_pool.tile([128, CT, HQ * 128], BF16, tag="w1h0r",
                                  name="w1h0r")
            nc.gpsimd.dma_start(out=w1[0],
                in_=wfc1T.rearrange("(ct p) f -> p ct f",
                                    p=128)[:, :, 0:HQ * 128])
            w2[0] = w2b_pool.tile([128, HQ, C], BF16, tag="w2h0r",
                                  name="w2h0r")
            nc.gpsimd.dma_start(out=w2[0],
                in_=wfc2T.rearrange("(ht p) f -> p ht f", p=128)[:, 0:HQ, :])
            h2c_apply(1)
            for hb, ch in ((1, 0), (0, 1), (1, 1)):
                for ht in range(HQ):
                    make_fc1(fpst, w1[hb], hb, ch, ht)()
                for ft in range(CT):
                    make_fc2(fpst, w2[hb], hb, ch, ft)()


def kernel(**inputs):
    global _NC_CACHE, LAST_RESULTS
    import os
    ins = {k: np.ascontiguousarray(np.asarray(v, dtype=np.float32))
           for k, v in inputs.items()}
    if _NC_CACHE is None:
        _NC_CACHE = build_program()
    nc = _NC_CACHE

    import ml_dtypes
    bf = ml_dtypes.bfloat16
    shared = {
        "wqkvT": np.ascontiguousarray(ins["qkv_w"].T.astype(bf)),
        "wprojT": np.ascontiguousarray(ins["proj_w"].T.astype(bf)),
        "wfc1T": np.ascontiguousarray(ins["fc1_w"].T.astype(bf)),
        "wfc2T": np.ascontiguousarray(ins["fc2_w"].T.astype(bf)),
        "pb": ins["proj_b"], "f1b": ins["fc1_b"], "f2b": ins["fc2_b"],
        "g1": ins["ln1_g"], "b1": ins["ln1_b"],
        "g2": ins["ln2_g"], "b2": ins["ln2_b"],
    }
    in_maps = []
    for s in range(NCORES):
        b, half = s // 2, s % 2
        m = dict(shared)
        m["xb"] = np.ascontiguousarray(np.roll(ins["x"][b], -half * NO, axis=0))
        in_maps.append(m)

    trace = bool(int(os.environ.get("KBENCH_TRACE", "0")))
    LAST_RESULTS = run_bass_kernel_spmd(
        nc, in_maps, core_ids=list(range(NCORES)), trace=trace)
    out = np.empty((B, N, C), np.float32)
    for s in range(NCORES):
        b, half = s // 2, s % 2
        out[b, half * NO:(half + 1) * NO, :] = LAST_RESULTS.results[s]["outT"].T
    return out


# revision 24
# speedup vs baseline: 1.2003x; 1.0565x over previous
"""Trainium2 Bass kernel for a dense transformer block (B=4, N=2048, C=768,
H=12, D=64, HID=3072), sharded over 8 NeuronCores.

Sharding: token-split, no collectives. Core s handles batch b = s//2,
sequence half = s%2 (1024 tokens). Each core receives its batch element's
full 2048-token x (rolled so its own tokens are rows 0..1023), computes
K/V over all 2048 tokens (redundantly with its pair core), and produces
the output for its own 1024 tokens. Host gathers/transposes.

Layout: activations are kept feature-major ("X^T", [C, tokens]) so every
linear layer is a natural PE matmul (weights pre-transposed on host).
Attention computes S^T = K^T-tiles.T @ Q^T per head with softmax along
the partition (key) axis; the two heads of a feature-tile pair run
concurrently on the PE via tile_position row packing and share ONE
[128,1024] 2-bank PSUM tile so a single paired Exp activation covers
both. Denominators come from a ones-column appended to V; normalization
via GPSIMD partition-broadcast.

Pipelining: the kernel is emission-order software-pipelined so the
ScalarE exp stream (the attention bottleneck) always has PE work
running underneath it, keeping the PE HAM clock warm:
  D: attention(chunk0) interleaved with K/Q generation for the next
     head pair.
  F: attention(chunk1) interleaved with the first MLP quarter
     (fc1+fc2 of hidden-half 0, chunk 0).
  H: remaining three MLP quarters back-to-back (PE dense).
QKV/attention/proj and the MLP run in bf16 (fp32 PSUM accumulation).
"""

import numpy as np

import concourse.bass as bass
import concourse.mybir as mybir
import concourse.tile as tile
from concourse import bacc
from concourse.bass_utils import run_bass_kernel_spmd
from concourse.masks import make_identity

F32 = mybir.dt.float32
BF16 = mybir.dt.bfloat16
AF = mybir.ActivationFunctionType
ALU = mybir.AluOpType

B, N, C = 4, 2048, 768
H, D = 12, 64
HID = 3072
EPS = 1e-5
NCORES = 8
NO = 1024  # tokens owned per core
NKV = 2048  # key/value tokens per core
CT = C // 128  # 6 feature tiles
HT = HID // 128  # 24 hidden tiles
HQ = HT // 2  # 12 hidden tiles per half
KT = NKV // 128  # 16 kv token tiles
QCH = NO // 512  # 2 query chunks of 512
ISCALE = 1.0 / np.sqrt(D)

LAST_RESULTS = None
_NC_CACHE = None


def build_program(repeats=1):
    nc = bacc.Bacc(trn_type="TRN2", target_bir_lowering=False, num_devices=NCORES)

    xb = nc.dram_tensor("xb", [NKV, C], F32, kind="ExternalInput").ap()
    wqkvT = nc.dram_tensor("wqkvT", [C, 3 * C], BF16, kind="ExternalInput").ap()
    wprojT = nc.dram_tensor("wprojT", [C, C], BF16, kind="ExternalInput").ap()
    wfc1T = nc.dram_tensor("wfc1T", [C, HID], BF16, kind="ExternalInput").ap()
    wfc2T = nc.dram_tensor("wfc2T", [HID, C], BF16, kind="ExternalInput").ap()
    pb = nc.dram_tensor("pb", [C], F32, kind="ExternalInput").ap()
    f1b = nc.dram_tensor("f1b", [HID], F32, kind="ExternalInput").ap()
    f2b = nc.dram_tensor("f2b", [C], F32, kind="ExternalInput").ap()
    g1 = nc.dram_tensor("g1", [C], F32, kind="ExternalInput").ap()
    b1 = nc.dram_tensor("b1", [C], F32, kind="ExternalInput").ap()
    g2 = nc.dram_tensor("g2", [C], F32, kind="ExternalInput").ap()
    b2 = nc.dram_tensor("b2", [C], F32, kind="ExternalInput").ap()
    outT = nc.dram_tensor("outT", [C, NO], F32, kind="ExternalOutput").ap()

    with tile.TileContext(nc) as tc:
        for _ in range(repeats):
            emit(nc, tc, xb, wqkvT, wprojT, wfc1T, wfc2T, pb, f1b, f2b,
                 g1, b1, g2, b2, outT)
    nc.compile()
    return nc


def emit(nc, tc, xb, wqkvT, wprojT, wfc1T, wfc2T, pb, f1b, f2b,
         g1, b1, g2, b2, outT):
    dma = nc.sync.dma_start
    from contextlib import ExitStack

    with ExitStack() as top:
        lpool = lambda name, bufs: top.enter_context(
            tc.tile_pool(name=name, bufs=bufs))
        rpool = lambda name, bufs: top.enter_context(
            tc.tile_pool(name=name, bufs=bufs, side="right"))
        # ---- left side: constants + attention-phase working set ----
        consts = lpool("consts", 1)
        x2T_pool = lpool("x2T", 1)
        oT_pool = lpool("oT", 1)
        p_sb = lpool("p_sb", 3)
        asm = lpool("attn_sm", 2)
        wsb = lpool("work_sb", 2)
        wp_pool = lpool("wproj", 1)
        # ---- right side: MLP-persistent pools (live into phase H) ----
        g_pool = rpool("g_sb", 1)
        h2_pool = rpool("h2c", 1)
        acc_pool = rpool("acc_sb", 1)
        osb = rpool("out_sb", 2)
        bc_pool = rpool("bc_sb", 1)
        l2s = rpool("ln2_sm", 1)

        # ---- constants ----
        ident = consts.tile([128, 128], F32, tag="ident")
        make_identity(nc, ident)
        ident_bf = consts.tile([128, 128], BF16, tag="ident_bf")
        make_identity(nc, ident_bf)
        ones_f32 = consts.tile([128, 1], F32, tag="ones_f32")
        nc.vector.memset(ones_f32, 1.0)
        ones_cb = consts.tile([128, 1], BF16, tag="ones_cb")
        nc.scalar.activation(out=ones_cb, in_=ones_f32, func=AF.Copy)
        eps_t = consts.tile([128, 1], F32, tag="eps")
        nc.vector.memset(eps_t, EPS)
        # per-feature vectors as [128, CT] (col ct = features ct*128..)
        g1_s = consts.tile([128, CT], F32, tag="g1")
        dma(out=g1_s, in_=g1.rearrange("(ct p) -> p ct", p=128))
        b1_s = consts.tile([128, CT], F32, tag="b1")
        dma(out=b1_s, in_=b1.rearrange("(ct p) -> p ct", p=128))
        g2_s = consts.tile([128, CT], F32, tag="g2")
        dma(out=g2_s, in_=g2.rearrange("(ct p) -> p ct", p=128))
        b2_s = consts.tile([128, CT], F32, tag="b2")
        dma(out=b2_s, in_=b2.rearrange("(ct p) -> p ct", p=128))
        pb_s = consts.tile([128, CT], F32, tag="pb")
        dma(out=pb_s, in_=pb.rearrange("(ct p) -> p ct", p=128))
        f2b_s = consts.tile([128, CT], F32, tag="f2b")
        dma(out=f2b_s, in_=f2b.rearrange("(ct p) -> p ct", p=128))
        f1b_s = consts.tile([128, HT], F32, tag="f1b")
        dma(out=f1b_s, in_=f1b.rearrange("(ht p) -> p ht", p=128))

        x2T = [x2T_pool.tile([128, NO], BF16, tag=f"x2T{ct}", name=f"x2T{ct}")
               for ct in range(CT)]
        oT = [oT_pool.tile([128, 512], BF16, tag=f"oT{ct}", name=f"oT{ct}")
              for ct in range(CT)]
        wp = wp_pool.tile([128, CT, C], BF16, tag="wproj")
        g_sb = [g_pool.tile([128, 512], BF16, tag=f"g{ht}", name=f"g{ht}")
                for ht in range(HQ)]
        h2c = [[h2_pool.tile([128, 512], BF16, tag=f"h2_{ct}_{ch}",
                             name=f"h2_{ct}_{ch}") for ct in range(CT)]
               for ch in range(QCH)]
        acc = [[acc_pool.tile([128, 512], BF16, tag=f"acc{ft}_{ch}",
                              name=f"acc{ft}_{ch}") for ch in range(QCH)]
               for ft in range(CT)]
        ln2v = [None, None]
        qs = (slice(0, 64), slice(64, 128))
        qT, kT, vA, hkvT = [], [], [], []

        def h2c_apply(ch):
            """LN2 apply for chunk ch: h2c = (x2T*rstd + nmr)*g2 + b2 (DVE)."""
            cs = slice(ch * 512, (ch + 1) * 512)
            rstd, nmr = ln2v[ch]
            bc_r = bc_pool.tile([128, 512], F32, tag="bc_r")
            nc.gpsimd.partition_broadcast(bc_r, rstd, channels=128)
            bc_m = bc_pool.tile([128, 512], F32, tag="bc_m")
            nc.gpsimd.partition_broadcast(bc_m, nmr, channels=128)
            for ct in range(CT):
                t = osb.tile([128, 512], F32, tag="h2tmp")
                nc.vector.tensor_mul(t, x2T[ct][:, cs], bc_r)
                t2 = osb.tile([128, 512], F32, tag="h2tmp")
                nc.vector.tensor_add(t2, t, bc_m)
                nc.vector.tensor_scalar(
                    out=h2c[ch][ct], in0=t2,
                    scalar1=g2_s[:, ct:ct + 1], op0=ALU.mult,
                    scalar2=b2_s[:, ct:ct + 1], op1=ALU.add)

        def make_fc1(pool, w1t, hb, ch, ht, defer_gelu=False):
            def f():
                ps = pool.tile([128, 512], F32, tag=pool._ktag)
                for ct in range(CT):
                    nc.tensor.matmul(
                        ps, w1t[:, ct, ht * 128:(ht + 1) * 128], h2c[ch][ct],
                        start=(ct == 0), stop=(ct == CT - 1))
                hti = hb * HQ + ht
                if defer_gelu:
                    # raw pre-activation out; gelu runs batched in phase H
                    # so the exp ACT table isn't thrashed mid-attention.
                    nc.vector.tensor_copy(out=g_sb[ht], in_=ps)
                else:
                    nc.scalar.activation(out=g_sb[ht], in_=ps, func=AF.Gelu,
                                         bias=f1b_s[:, hti:hti + 1],
                                         scale=1.0)
            return f

        def make_fc2(pool, w2t, hb, ch, ft, gs=None):
            def f():
                g_src = gs if gs is not None else g_sb
                cs = slice(ch * 512, (ch + 1) * 512)
                ps = pool.tile([128, 512], F32, tag=pool._ktag)
                for ht in range(HQ):
                    nc.tensor.matmul(
                        ps, w2t[:, ht, ft * 128:(ft + 1) * 128], g_src[ht],
                        start=(ht == 0), stop=(ht == HQ - 1))
                if hb == 0:
                    nc.vector.scalar_tensor_tensor(
                        out=acc[ft][ch], in0=ps,
                        scalar=f2b_s[:, ft:ft + 1], in1=x2T[ft][:, cs],
                        op0=ALU.add, op1=ALU.add)
                else:
                    ot = osb.tile([128, 512], F32, tag="ot")
                    nc.vector.tensor_add(ot, ps, acc[ft][ch])
                    dma(out=outT[ft * 128:(ft + 1) * 128, cs], in_=ot)
            return f

        def proj_ln2(ch, pool, tag):
            """proj + residual into x2T (in place), then LN2 stats."""
            cs = slice(ch * 512, (ch + 1) * 512)
            for ft in range(CT):
                ps = pool.tile([128, 512], F32, tag=tag, name="ppj")
                for ct in range(CT):
                    nc.tensor.matmul(
                        ps, wp[:, ct, ft * 128:(ft + 1) * 128], oT[ct],
                        start=(ct == 0), stop=(ct == CT - 1))
                nc.vector.scalar_tensor_tensor(
                    out=x2T[ft][:, cs], in0=ps, scalar=pb_s[:, ft:ft + 1],
                    in1=x2T[ft][:, cs], op0=ALU.add, op1=ALU.add)
            psum = pool.tile([1, 512], F32, tag=tag, name="psum")
            pssq = pool.tile([1, 512], F32, tag=tag, name="pssq")
            for ct in range(CT):
                sq = wsb.tile([128, 512], BF16, tag="sq", bufs=1)
                nc.vector.tensor_mul(sq, x2T[ct][:, cs], x2T[ct][:, cs])
                nc.tensor.matmul(psum, ones_cb, x2T[ct][:, cs],
                                 start=(ct == 0), stop=(ct == CT - 1),
                                 skip_group_check=True)
                nc.tensor.matmul(pssq, ones_cb, sq,
                                 start=(ct == 0), stop=(ct == CT - 1),
                                 skip_group_check=True)
            mu = l2s.tile([1, 512], F32, tag="mu")
            nc.scalar.mul(mu, psum, 1.0 / C)
            msq = l2s.tile([1, 512], F32, tag="msq")
            nc.scalar.mul(msq, pssq, 1.0 / C)
            mu2 = l2s.tile([1, 512], F32, tag="tmpa")
            nc.vector.tensor_mul(mu2, mu, mu)
            # var in place over msq; rstd = exp(-0.5*ln(var+eps)) keeps the
            # ACT table in the natural_log_exp set (no Sqrt-set thrash
            # between the attention exp streams).
            nc.vector.tensor_sub(msq, msq, mu2)
            nc.scalar.activation(out=mu2, in_=msq, func=AF.Ln,
                                 bias=eps_t[0:1], scale=1.0)
            rstd = l2s.tile([1, 512], F32, tag=f"rstd2_{ch}", name="rstd2")
            nc.scalar.activation(out=rstd, in_=mu2, func=AF.Exp,
                                 scale=-0.5)
            nmr = l2s.tile([1, 512], F32, tag=f"nmr2_{ch}", name="nmr2")
            nc.vector.scalar_tensor_tensor(
                out=nmr, in0=mu, scalar=-1.0, in1=rstd,
                op0=ALU.mult, op1=ALU.mult)
            ln2v[ch] = (rstd, nmr)

        with (
            tc.tile_pool(name="sp_psum", bufs=2, space="PSUM") as sps,
            tc.tile_pool(name="o_psum", bufs=2, space="PSUM") as ops,
        ):
            def attn_hp(hp, ch):
                """Attention for head pair hp on query chunk ch."""
                qch = slice(ch * 512, (ch + 1) * 512)

                def s_pair(nt):
                    ps = sps.tile([128, 1024], F32, tag="sp", name="spS")
                    for i in range(2):
                        nc.tensor.matmul(
                            ps[:, i * 512:(i + 1) * 512],
                            kT[hp][qs[i], nt * 128:(nt + 1) * 128],
                            qT[hp][qs[i], qch],
                            start=True, stop=True,
                            tile_position=(64 * i, 0))
                    return ps

                po = [ops.tile([D + 1, 512], F32, tag="po", name="po")
                      for _ in range(2)]
                ps_cur = s_pair(0)
                for nt in range(KT):
                    ps_next = s_pair(nt + 1) if nt < KT - 1 else None
                    pt = p_sb.tile([128, 1024], BF16, tag="pt")
                    nc.scalar.activation(out=pt, in_=ps_cur,
                                         func=AF.Exp, scale=ISCALE)
                    for i in range(2):
                        nc.tensor.matmul(
                            po[i], vA[nt][:, 2 * hp + i, :],
                            pt[:, i * 512:(i + 1) * 512],
                            start=(nt == 0), stop=(nt == KT - 1),
                            skip_group_check=True)
                    ps_cur = ps_next
                for i in range(2):
                    # 1/den = exp(-ln(den)): Ln/Exp share the attention
                    # exp's ACT table set, so this slots into the exp
                    # stream with no table reload and no DVE reciprocal.
                    lden = asm.tile([1, 512], F32, tag="lden")
                    nc.scalar.activation(out=lden, in_=po[i][D:D + 1, :],
                                         func=AF.Ln)
                    rec = asm.tile([1, 512], BF16, tag="rec")
                    nc.scalar.activation(out=rec, in_=lden, func=AF.Exp,
                                         scale=-1.0)
                    vb = asm.tile([D, 512], BF16, tag="vb")
                    nc.gpsimd.partition_broadcast(vb, rec, channels=D)
                    nc.vector.tensor_mul(
                        oT[hp][qs[i], :], po[i][0:D, :], vb)

            def kq_unit(ft):
                for chh in range(NKV // 512):
                    ps = emit.mmq.tile([128, 512], F32, tag="mmq")
                    for ct in range(CT):
                        nc.tensor.matmul(
                            ps, wqk[:, ct, C + ft * 128:C + (ft + 1) * 128],
                            hkvT[ct][:, chh * 512:(chh + 1) * 512],
                            start=(ct == 0), stop=(ct == CT - 1))
                    nc.vector.tensor_copy(
                        out=kT[ft][:, chh * 512:(chh + 1) * 512], in_=ps)
                for chh in range(QCH):
                    ps = emit.mmq.tile([128, 512], F32, tag="mmq")
                    for ct in range(CT):
                        nc.tensor.matmul(
                            ps, wqk[:, ct, ft * 128:(ft + 1) * 128],
                            hkvT[ct][:, chh * 512:(chh + 1) * 512],
                            start=(ct == 0), stop=(ct == CT - 1))
                    nc.vector.tensor_copy(
                        out=qT[ft][:, chh * 512:(chh + 1) * 512], in_=ps)

            # ============ Phase A: LN1, transposes ============
            kv_stack = ExitStack()
            hkvT_pool = kv_stack.enter_context(
                tc.tile_pool(name="hkvT", bufs=1, side="right"))
            wqk_pool = kv_stack.enter_context(
                tc.tile_pool(name="wqkv_kq", bufs=1, side="right"))
            wv_stack = ExitStack()
            wqv_pool = wv_stack.enter_context(
                tc.tile_pool(name="wqkv_v", bufs=1, side="right"))
            hkvT.extend(hkvT_pool.tile([128, NKV], BF16, tag=f"hkvT{ct}",
                                       name=f"hkvT{ct}") for ct in range(CT))
            wqk = wqk_pool.tile([128, CT, 2 * C], BF16, tag="wqkv_kq")
            wqv = wqv_pool.tile([128, CT, C], BF16, tag="wqkv_v")
            with (
                tc.tile_pool(name="ln1_work", bufs=2) as lw,
                tc.tile_pool(name="ln1_stat", bufs=6) as lstat,
            ):
                for g in range(KT // 4):  # groups of 4 token tiles
                    xts, xcs = [], []
                    for j in range(4):
                        nt = 4 * g + j
                        xt = lw.tile([128, C], F32, tag=f"xt{j}",
                                     name=f"xt{j}")
                        dma(out=xt, in_=xb[nt * 128:(nt + 1) * 128, :])
                        st = lstat.tile([128, 3, 6], F32, tag="st")
                        xg = xt.rearrange("p (s d) -> p s d", s=3)
                        for s in range(3):
                            nc.vector.bn_stats(out=st[:, s], in_=xg[:, s])
                        mv = lstat.tile([128, 2], F32, tag="mv")
                        nc.vector.bn_aggr(out=mv, in_=st)
                        rstd = lstat.tile([128, 1], F32, tag="rstd")
                        nc.scalar.activation(out=rstd, in_=mv[:, 1:2],
                                             func=AF.Sqrt,
                                             bias=eps_t, scale=1.0)
                        nc.vector.reciprocal(out=rstd, in_=rstd)
                        nmr = lstat.tile([128, 1], F32, tag="nmr")
                        nc.vector.tensor_scalar(
                            out=nmr, in0=mv[:, 0:1], scalar1=-1.0,
                            scalar2=rstd, op0=ALU.mult, op1=ALU.mult)
                        xc = lw.tile([128, C], BF16, tag=f"xc{j}",
                                     name=f"xc{j}", bufs=1)
                        nc.scalar.activation(out=xc, in_=xt,
                                             func=AF.Identity,
                                             scale=rstd, bias=nmr)
                        xts.append(xt)
                        xcs.append(xc)
                    if g == 0:
                        # land in the DMA queue before QKV matmuls need them
                        nc.gpsimd.dma_start(out=wqv,
                            in_=wqkvT.rearrange("(ct p) f -> p ct f",
                                                p=128)[:, :, 2 * C:3 * C])
                        nc.gpsimd.dma_start(out=wqk,
                            in_=wqkvT.rearrange("(ct p) f -> p ct f",
                                                p=128)[:, :, 0:2 * C])
                    for ct in range(CT):
                        ps = sps.tile([128, 1024], BF16, tag="sp",
                                      name="ptr")
                        for j in range(4):
                            nc.tensor.transpose(
                                ps[:, j * 128:(j + 1) * 128],
                                xcs[j][:, ct * 128:(ct + 1) * 128],
                                ident_bf)
                        nc.scalar.activation(
                            out=hkvT[ct][:, g * 512:(g + 1) * 512],
                            in_=ps[:, 0:512], func=AF.Identity,
                            scale=g1_s[:, ct:ct + 1], bias=b1_s[:, ct:ct + 1])
                    if g < NO // 512:  # own tokens: raw x^T for residual
                        for ct in range(CT):
                            ps32 = sps.tile([128, 1024], F32, tag="sp",
                                            name="ptr32")
                            for j in range(4):
                                nc.tensor.transpose(
                                    ps32[:, j * 128:(j + 1) * 128],
                                    xts[j][:, ct * 128:(ct + 1) * 128],
                                    ident)
                            nc.vector.tensor_copy(
                                out=x2T[ct][:, g * 512:(g + 1) * 512],
                                in_=ps32[:, 0:512])
                # proj weights: queue behind x/wq so LN1 isn't delayed
                nc.gpsimd.dma_start(out=wp,
                    in_=wprojT.rearrange("(ct p) f -> p ct f", p=128))

            # ============ Phase B: V ============
            qkv_stack = ExitStack()
            emit.qkv_stack = qkv_stack
            vA_pool = qkv_stack.enter_context(
                tc.tile_pool(name="vA", bufs=1))
            vA.extend(vA_pool.tile([128, H, D + 1], BF16, tag=f"vA{nt}",
                                   name=f"vA{nt}") for nt in range(KT))
            for nt in range(KT):
                psv = sps.tile([128, 1024], F32, tag="sp", name="psv")
                for ct in range(CT):
                    hk = hkvT[ct][:, nt * 128:(nt + 1) * 128]
                    nc.tensor.matmul(psv[:, 0:512], hk,
                                     wqv[:, ct, 0:512],
                                     start=(ct == 0), stop=(ct == CT - 1))
                    nc.tensor.matmul(psv[:, 512:768], hk,
                                     wqv[:, ct, 512:C],
                                     start=(ct == 0), stop=(ct == CT - 1))
                nc.vector.tensor_copy(
                    out=vA[nt][:, 0:8, 0:D],
                    in_=psv[:, 0:512].rearrange("p (h d) -> p h d", d=D))
                nc.vector.tensor_copy(
                    out=vA[nt][:, 8:12, 0:D],
                    in_=psv[:, 512:768].rearrange("p (h d) -> p h d", d=D))
                nc.vector.memset(vA[nt][:, :, D:D + 1], 1.0)
            wv_stack.close()  # wqv freed

            # ===== Phases C/D/E: K/Q + attention(ch0) + proj/LN2(ch0) =====
            qT_pool = qkv_stack.enter_context(
                tc.tile_pool(name="qT", bufs=1))
            kT_pool = qkv_stack.enter_context(
                tc.tile_pool(name="kT", bufs=1))
            qT.extend(qT_pool.tile([128, NO], BF16, tag=f"qT{ct}",
                                   name=f"qT{ct}") for ct in range(CT))
            kT.extend(kT_pool.tile([128, NKV], BF16, tag=f"kT{ct}",
                                   name=f"kT{ct}") for ct in range(CT))
            with tc.tile_pool(name="mmq_psum", bufs=2, space="PSUM") as mmq:
                emit.mmq = mmq
                kq_unit(0)
                for hp in range(CT):
                    if hp + 1 < CT:
                        kq_unit(hp + 1)
                    attn_hp(hp, 0)
                proj_ln2(0, mmq, "mmq")
                h2c_apply(0)
            kv_stack.close()  # hkvT + K/Q weights freed

            # ===== Phase F: attention(ch1) + MLP quarter (hb0, ch0) =====
            with (
                tc.tile_pool(name="wfc1f", bufs=1, side="right") as w1f_pool,
                tc.tile_pool(name="wfc2f", bufs=1, side="right") as w2f_pool,
                tc.tile_pool(name="f_psum", bufs=2, space="PSUM") as fps,
            ):
                fps._ktag = "f1"
                w1f = w1f_pool.tile([128, CT, HQ * 128], BF16, tag="w1f")
                nc.gpsimd.dma_start(out=w1f,
                    in_=wfc1T.rearrange("(ct p) f -> p ct f",
                                        p=128)[:, :, 0:HQ * 128])
                filler = [
                    [],
                    [make_fc1(fps, w1f, 0, 0, ht, defer_gelu=True)
                     for ht in range(3)],
                    [make_fc1(fps, w1f, 0, 0, ht, defer_gelu=True)
                     for ht in range(3, 6)],
                    [make_fc1(fps, w1f, 0, 0, ht, defer_gelu=True)
                     for ht in range(6, 9)],
                    [make_fc1(fps, w1f, 0, 0, ht, defer_gelu=True)
                     for ht in range(9, HQ)],
                    [],
                ]
                for hp in range(CT):
                    for u in filler[hp]:
                        u()
                    attn_hp(hp, 1)
                proj_ln2(1, fps, "f1")
        emit.qkv_stack.close()  # qT/kT/vA freed

        # ======= Phases G/H: LN2 apply ch1 + remaining MLP quarters =======
        with (
            tc.tile_pool(name="wfc1b", bufs=1, side="right") as w1b_pool,
            tc.tile_pool(name="wfc2b", bufs=1, side="right") as w2b_pool,
            tc.tile_pool(name="ft_psum", bufs=4, space="PSUM") as fpst,
        ):
            fpst._ktag = "ft"
            w1 = {}
            w2 = {}
            w2[0] = w2b_pool.tile([128, HQ, C], BF16, tag="w2h0r",
                                  name="w2h0r")
            nc.gpsimd.dma_start(out=w2[0],
                in_=wfc2T.rearrange("(ht p) f -> p ht f", p=128)[:, 0:HQ, :])
            w1[1] = w1b_pool.tile([128, CT, HQ * 128], BF16, tag="w1h1",
                                  name="w1h1")
            nc.gpsimd.dma_start(out=w1[1],
                in_=wfc1T.rearrange("(ct p) f -> p ct f",
                                    p=128)[:, :, HQ * 128:HID])
            w2[1] = w2b_pool.tile([128, HQ, C], BF16, tag="w2h1",
                                  name="w2h1")
            nc.gpsimd.dma_start(out=w2[1],
                in_=wfc2T.rearrange("(ht p) f -> p ht f", p=128)[:, HQ:HT, :])
            w1[0] = w1b_pool.tile([128, CT, HQ * 128], BF16, tag="w1h0r",
                                  name="w1h0r")
            nc.gpsimd.dma_start(out=w1[0],
                in_=wfc1T.rearrange("(ct p) f -> p ct f",
                                    p=128)[:, :, 0:HQ * 128])
            h2c_apply(1)
            # batched gelu for the F-phase fc1 quarter (one table load),
            # then its fc2 -> acc; then the remaining three quarters.
            gg = [w1b_pool.tile([128, 512], BF16, tag=f"gg{ht}",
                                name=f"gg{ht}") for ht in range(HQ)]
            for ht in range(HQ):
                nc.scalar.activation(out=gg[ht], in_=g_sb[ht], func=AF.Gelu,
                                     bias=f1b_s[:, ht:ht + 1], scale=1.0)
            for ft in range(CT):
                make_fc2(fpst, w2[0], 0, 0, ft, gs=gg)()
            for hb, ch in ((1, 0), (0, 1), (1, 1)):
                for ht in range(HQ):
                    make_fc1(fpst, w1[hb], hb, ch, ht)()
                for ft in range(CT):
                    make_fc2(fpst, w2[hb], hb, ch, ft)()


def kernel(**inputs):
    global _NC_CACHE, LAST_RESULTS
    import os
    ins = {k: np.ascontiguousarray(np.asarray(v, dtype=np.float32))
           for k, v in inputs.items()}
    if _NC_CACHE is None:
        _NC_CACHE = build_program()
    nc = _NC_CACHE

    import ml_dtypes
    bf = ml_dtypes.bfloat16
    shared = {
        "wqkvT": np.ascontiguousarray(ins["qkv_w"].T.astype(bf)),
        "wprojT": np.ascontiguousarray(ins["proj_w"].T.astype(bf)),
        "wfc1T": np.ascontiguousarray(ins["fc1_w"].T.astype(bf)),
        "wfc2T": np.ascontiguousarray(ins["fc2_w"].T.astype(bf)),
        "pb": ins["proj_b"], "f1b": ins["fc1_b"], "f2b": ins["fc2_b"],
        "g1": ins["ln1_g"], "b1": ins["ln1_b"],
        "g2": ins["ln2_g"], "b2": ins["ln2_b"],
    }
    in_maps = []
    for s in range(NCORES):
        b, half = s // 2, s % 2
        m = dict(shared)
        m["xb"] = np.ascontiguousarray(np.roll(ins["x"][b], -half * NO, axis=0))
        in_maps.append(m)

    trace = bool(int(os.environ.get("KBENCH_TRACE", "0")))
    LAST_RESULTS = run_bass_kernel_spmd(
        nc, in_maps, core_ids=list(range(NCORES)), trace=trace)
    out = np.empty((B, N, C), np.float32)
    for s in range(NCORES):
        b, half = s // 2, s % 2
        out[b, half * NO:(half + 1) * NO, :] = LAST_RESULTS.results[s]["outT"].T
    return out


# revision 25
# speedup vs baseline: 1.2433x; 1.0358x over previous
"""Trainium2 Bass kernel for a dense transformer block (B=4, N=2048, C=768,
H=12, D=64, HID=3072), sharded over 8 NeuronCores.

Sharding: token-split, no collectives. Core s handles batch b = s//2,
sequence half = s%2 (1024 tokens). Each core receives its batch element's
full 2048-token x (rolled so its own tokens are rows 0..1023), computes
K/V over all 2048 tokens (redundantly with its pair core), and produces
the output for its own 1024 tokens. Host gathers/transposes.

Layout: activations are kept feature-major ("X^T", [C, tokens]) so every
linear layer is a natural PE matmul (weights pre-transposed on host).
Attention computes S^T = K^T-tiles.T @ Q^T per head with softmax along
the partition (key) axis; the two heads of a feature-tile pair run
concurrently on the PE via tile_position row packing and share ONE
[128,1024] 2-bank PSUM tile so a single paired Exp activation covers
both. Denominators come from a ones-column appended to V; normalization
via GPSIMD partition-broadcast.

Pipelining: the kernel is emission-order software-pipelined so the
ScalarE exp stream (the attention bottleneck) always has PE work
running underneath it, keeping the PE HAM clock warm:
  D: attention(chunk0) interleaved with K/Q generation for the next
     head pair.
  F: attention(chunk1) interleaved with the first MLP quarter
     (fc1+fc2 of hidden-half 0, chunk 0).
  H: remaining three MLP quarters back-to-back (PE dense).
QKV/attention/proj and the MLP run in bf16 (fp32 PSUM accumulation).
"""

import numpy as np

import concourse.bass as bass
import concourse.mybir as mybir
import concourse.tile as tile
from concourse import bacc
from concourse.bass_utils import run_bass_kernel_spmd
from concourse.masks import make_identity

F32 = mybir.dt.float32
BF16 = mybir.dt.bfloat16
AF = mybir.ActivationFunctionType
ALU = mybir.AluOpType

B, N, C = 4, 2048, 768
H, D = 12, 64
HID = 3072
EPS = 1e-5
NCORES = 8
NO = 1024  # tokens owned per core
NKV = 2048  # key/value tokens per core
CT = C // 128  # 6 feature tiles
HT = HID // 128  # 24 hidden tiles
HQ = HT // 2  # 12 hidden tiles per half
KT = NKV // 128  # 16 kv token tiles
QCH = NO // 512  # 2 query chunks of 512
ISCALE = 1.0 / np.sqrt(D)

LAST_RESULTS = None
_NC_CACHE = None


def build_program(repeats=1):
    nc = bacc.Bacc(trn_type="TRN2", target_bir_lowering=False, num_devices=NCORES)

    xb = nc.dram_tensor("xb", [NKV, C], F32, kind="ExternalInput").ap()
    wqkvT = nc.dram_tensor("wqkvT", [C, 3 * C], BF16, kind="ExternalInput").ap()
    wprojT = nc.dram_tensor("wprojT", [C, C], BF16, kind="ExternalInput").ap()
    wfc1T = nc.dram_tensor("wfc1T", [C, HID], BF16, kind="ExternalInput").ap()
    wfc2T = nc.dram_tensor("wfc2T", [HID, C], BF16, kind="ExternalInput").ap()
    pb = nc.dram_tensor("pb", [C], F32, kind="ExternalInput").ap()
    f1b = nc.dram_tensor("f1b", [HID], F32, kind="ExternalInput").ap()
    f2b = nc.dram_tensor("f2b", [C], F32, kind="ExternalInput").ap()
    g1 = nc.dram_tensor("g1", [C], F32, kind="ExternalInput").ap()
    b1 = nc.dram_tensor("b1", [C], F32, kind="ExternalInput").ap()
    g2 = nc.dram_tensor("g2", [C], F32, kind="ExternalInput").ap()
    b2 = nc.dram_tensor("b2", [C], F32, kind="ExternalInput").ap()
    outT = nc.dram_tensor("outT", [C, NO], F32, kind="ExternalOutput").ap()

    with tile.TileContext(nc) as tc:
        for _ in range(repeats):
            emit(nc, tc, xb, wqkvT, wprojT, wfc1T, wfc2T, pb, f1b, f2b,
                 g1, b1, g2, b2, outT)
    nc.compile()
    return nc


def emit(nc, tc, xb, wqkvT, wprojT, wfc1T, wfc2T, pb, f1b, f2b,
         g1, b1, g2, b2, outT):
    dma = nc.sync.dma_start
    from contextlib import ExitStack

    with ExitStack() as top:
        lpool = lambda name, bufs: top.enter_context(
            tc.tile_pool(name=name, bufs=bufs))
        rpool = lambda name, bufs: top.enter_context(
            tc.tile_pool(name=name, bufs=bufs, side="right"))
        # ---- left side: constants + attention-phase working set ----
        consts = lpool("consts", 1)
        x2T_pool = lpool("x2T", 1)
        oT_pool = lpool("oT", 1)
        p_sb = lpool("p_sb", 3)
        asm = lpool("attn_sm", 2)
        wsb = lpool("work_sb", 2)
        wp_pool = lpool("wproj", 1)
        # ---- right side: MLP-persistent pools (live into phase H) ----
        g_pool = rpool("g_sb", 1)
        h2_pool = rpool("h2c", 1)
        acc_pool = rpool("acc_sb", 1)
        osb = rpool("out_sb", 2)
        bc_pool = rpool("bc_sb", 1)
        l2s = rpool("ln2_sm", 1)

        # ---- constants ----
        ident = consts.tile([128, 128], F32, tag="ident")
        make_identity(nc, ident)
        ident_bf = consts.tile([128, 128], BF16, tag="ident_bf")
        make_identity(nc, ident_bf)
        ones_f32 = consts.tile([128, 1], F32, tag="ones_f32")
        nc.vector.memset(ones_f32, 1.0)
        ones_cb = consts.tile([128, 1], BF16, tag="ones_cb")
        nc.scalar.activation(out=ones_cb, in_=ones_f32, func=AF.Copy)
        eps_t = consts.tile([128, 1], F32, tag="eps")
        nc.vector.memset(eps_t, EPS)
        # per-feature vectors as [128, CT] (col ct = features ct*128..)
        g1_s = consts.tile([128, CT], F32, tag="g1")
        dma(out=g1_s, in_=g1.rearrange("(ct p) -> p ct", p=128))
        b1_s = consts.tile([128, CT], F32, tag="b1")
        dma(out=b1_s, in_=b1.rearrange("(ct p) -> p ct", p=128))
        g2_s = consts.tile([128, CT], F32, tag="g2")
        dma(out=g2_s, in_=g2.rearrange("(ct p) -> p ct", p=128))
        b2_s = consts.tile([128, CT], F32, tag="b2")
        dma(out=b2_s, in_=b2.rearrange("(ct p) -> p ct", p=128))
        pb_s = consts.tile([128, CT], F32, tag="pb")
        dma(out=pb_s, in_=pb.rearrange("(ct p) -> p ct", p=128))
        f2b_s = consts.tile([128, CT], F32, tag="f2b")
        dma(out=f2b_s, in_=f2b.rearrange("(ct p) -> p ct", p=128))
        f1b_s = consts.tile([128, HT], F32, tag="f1b")
        dma(out=f1b_s, in_=f1b.rearrange("(ht p) -> p ht", p=128))

        x2T = [x2T_pool.tile([128, NO], BF16, tag=f"x2T{ct}", name=f"x2T{ct}")
               for ct in range(CT)]
        oT = [oT_pool.tile([128, 512], BF16, tag=f"oT{ct}", name=f"oT{ct}")
              for ct in range(CT)]
        wp = wp_pool.tile([128, CT, C], BF16, tag="wproj")
        g_sb = [g_pool.tile([128, 512], BF16, tag=f"g{ht}", name=f"g{ht}")
                for ht in range(HQ)]
        h2c = [[h2_pool.tile([128, 512], BF16, tag=f"h2_{ct}_{ch}",
                             name=f"h2_{ct}_{ch}") for ct in range(CT)]
               for ch in range(QCH)]
        acc = [[acc_pool.tile([128, 512], BF16, tag=f"acc{ft}_{ch}",
                              name=f"acc{ft}_{ch}") for ch in range(QCH)]
               for ft in range(CT)]
        ln2v = [None, None]
        qs = (slice(0, 64), slice(64, 128))
        qT, kT, vA, hkvT = [], [], [], []

        def h2c_apply(ch):
            """LN2 apply for chunk ch: h2c = (x2T*rstd + nmr)*g2 + b2 (DVE)."""
            cs = slice(ch * 512, (ch + 1) * 512)
            rstd, nmr = ln2v[ch]
            bc_r = bc_pool.tile([128, 512], F32, tag="bc_r")
            nc.gpsimd.partition_broadcast(bc_r, rstd, channels=128)
            bc_m = bc_pool.tile([128, 512], F32, tag="bc_m")
            nc.gpsimd.partition_broadcast(bc_m, nmr, channels=128)
            for ct in range(CT):
                t = osb.tile([128, 512], F32, tag="h2tmp")
                nc.vector.tensor_mul(t, x2T[ct][:, cs], bc_r)
                t2 = osb.tile([128, 512], F32, tag="h2tmp")
                nc.vector.tensor_add(t2, t, bc_m)
                nc.vector.tensor_scalar(
                    out=h2c[ch][ct], in0=t2,
                    scalar1=g2_s[:, ct:ct + 1], op0=ALU.mult,
                    scalar2=b2_s[:, ct:ct + 1], op1=ALU.add)

        def make_fc1(pool, w1t, hb, ch, ht, defer_gelu=False):
            def f():
                ps = pool.tile([128, 512], F32, tag=pool._ktag)
                for ct in range(CT):
                    nc.tensor.matmul(
                        ps, w1t[:, ct, ht * 128:(ht + 1) * 128], h2c[ch][ct],
                        start=(ct == 0), stop=(ct == CT - 1))
                hti = hb * HQ + ht
                if defer_gelu:
                    # raw pre-activation out; gelu runs batched in phase H
                    # so the exp ACT table isn't thrashed mid-attention.
                    nc.vector.tensor_copy(out=g_sb[ht], in_=ps)
                else:
                    nc.scalar.activation(out=g_sb[ht], in_=ps, func=AF.Gelu,
                                         bias=f1b_s[:, hti:hti + 1],
                                         scale=1.0)
            return f

        def make_fc2(pool, w2t, hb, ch, ft, gs=None):
            def f():
                g_src = gs if gs is not None else g_sb
                cs = slice(ch * 512, (ch + 1) * 512)
                ps = pool.tile([128, 512], F32, tag=pool._ktag)
                for ht in range(HQ):
                    nc.tensor.matmul(
                        ps, w2t[:, ht, ft * 128:(ft + 1) * 128], g_src[ht],
                        start=(ht == 0), stop=(ht == HQ - 1))
                if hb == 0:
                    nc.vector.scalar_tensor_tensor(
                        out=acc[ft][ch], in0=ps,
                        scalar=f2b_s[:, ft:ft + 1], in1=x2T[ft][:, cs],
                        op0=ALU.add, op1=ALU.add)
                else:
                    ot = osb.tile([128, 512], F32, tag="ot")
                    nc.vector.tensor_add(ot, ps, acc[ft][ch])
                    dma(out=outT[ft * 128:(ft + 1) * 128, cs], in_=ot)
            return f

        def proj_ln2(ch, pool, tag):
            """proj + residual into x2T (in place), then LN2 stats."""
            cs = slice(ch * 512, (ch + 1) * 512)
            for ft in range(CT):
                ps = pool.tile([128, 512], F32, tag=tag, name="ppj")
                for ct in range(CT):
                    nc.tensor.matmul(
                        ps, wp[:, ct, ft * 128:(ft + 1) * 128], oT[ct],
                        start=(ct == 0), stop=(ct == CT - 1))
                nc.vector.scalar_tensor_tensor(
                    out=x2T[ft][:, cs], in0=ps, scalar=pb_s[:, ft:ft + 1],
                    in1=x2T[ft][:, cs], op0=ALU.add, op1=ALU.add)
            psum = pool.tile([1, 512], F32, tag=tag, name="psum")
            pssq = pool.tile([1, 512], F32, tag=tag, name="pssq")
            for ct in range(CT):
                sq = wsb.tile([128, 512], BF16, tag="sq", bufs=1)
                nc.vector.tensor_mul(sq, x2T[ct][:, cs], x2T[ct][:, cs])
                nc.tensor.matmul(psum, ones_cb, x2T[ct][:, cs],
                                 start=(ct == 0), stop=(ct == CT - 1),
                                 skip_group_check=True)
                nc.tensor.matmul(pssq, ones_cb, sq,
                                 start=(ct == 0), stop=(ct == CT - 1),
                                 skip_group_check=True)
            mu = l2s.tile([1, 512], F32, tag="mu")
            nc.scalar.mul(mu, psum, 1.0 / C)
            msq = l2s.tile([1, 512], F32, tag="msq")
            nc.scalar.mul(msq, pssq, 1.0 / C)
            mu2 = l2s.tile([1, 512], F32, tag="tmpa")
            nc.vector.tensor_mul(mu2, mu, mu)
            # var in place over msq; rstd = exp(-0.5*ln(var+eps)) keeps the
            # ACT table in the natural_log_exp set (no Sqrt-set thrash
            # between the attention exp streams).
            nc.vector.tensor_sub(msq, msq, mu2)
            nc.scalar.activation(out=mu2, in_=msq, func=AF.Ln,
                                 bias=eps_t[0:1], scale=1.0)
            rstd = l2s.tile([1, 512], F32, tag=f"rstd2_{ch}", name="rstd2")
            nc.scalar.activation(out=rstd, in_=mu2, func=AF.Exp,
                                 scale=-0.5)
            nmr = l2s.tile([1, 512], F32, tag=f"nmr2_{ch}", name="nmr2")
            nc.vector.scalar_tensor_tensor(
                out=nmr, in0=mu, scalar=-1.0, in1=rstd,
                op0=ALU.mult, op1=ALU.mult)
            ln2v[ch] = (rstd, nmr)

        with (
            tc.tile_pool(name="sp_psum", bufs=2, space="PSUM") as sps,
            tc.tile_pool(name="o_psum", bufs=2, space="PSUM") as ops,
        ):
            def attn_hp(hp, ch):
                """Attention for head pair hp on query chunk ch."""
                qch = slice(ch * 512, (ch + 1) * 512)

                def s_pair(nt):
                    ps = sps.tile([128, 1024], F32, tag="sp", name="spS")
                    for i in range(2):
                        nc.tensor.matmul(
                            ps[:, i * 512:(i + 1) * 512],
                            kT[hp][qs[i], nt * 128:(nt + 1) * 128],
                            qT[hp][qs[i], qch],
                            start=True, stop=True,
                            tile_position=(64 * i, 0))
                    return ps

                po = [ops.tile([D + 1, 512], F32, tag="po", name="po")
                      for _ in range(2)]
                ps_cur = s_pair(0)
                for nt in range(KT):
                    ps_next = s_pair(nt + 1) if nt < KT - 1 else None
                    pt = p_sb.tile([128, 1024], BF16, tag="pt")
                    nc.scalar.activation(out=pt, in_=ps_cur,
                                         func=AF.Exp, scale=ISCALE)
                    for i in range(2):
                        nc.tensor.matmul(
                            po[i], vA[nt][:, 2 * hp + i, :],
                            pt[:, i * 512:(i + 1) * 512],
                            start=(nt == 0), stop=(nt == KT - 1),
                            skip_group_check=True)
                    ps_cur = ps_next
                for i in range(2):
                    # 1/den = exp(-ln(den)): Ln/Exp share the attention
                    # exp's ACT table set, so this slots into the exp
                    # stream with no table reload and no DVE reciprocal.
                    lden = asm.tile([1, 512], F32, tag="lden")
                    nc.scalar.activation(out=lden, in_=po[i][D:D + 1, :],
                                         func=AF.Ln)
                    rec = asm.tile([1, 512], BF16, tag="rec")
                    nc.scalar.activation(out=rec, in_=lden, func=AF.Exp,
                                         scale=-1.0)
                    vb = asm.tile([D, 512], BF16, tag="vb")
                    nc.gpsimd.partition_broadcast(vb, rec, channels=D)
                    nc.vector.tensor_mul(
                        oT[hp][qs[i], :], po[i][0:D, :], vb)

            def kq_unit(ft):
                for chh in range(NKV // 512):
                    ps = emit.mmq.tile([128, 512], F32, tag="mmq")
                    for ct in range(CT):
                        nc.tensor.matmul(
                            ps, wqk[:, ct, C + ft * 128:C + (ft + 1) * 128],
                            hkvT[ct][:, chh * 512:(chh + 1) * 512],
                            start=(ct == 0), stop=(ct == CT - 1))
                    nc.vector.tensor_copy(
                        out=kT[ft][:, chh * 512:(chh + 1) * 512], in_=ps)
                for chh in range(QCH):
                    ps = emit.mmq.tile([128, 512], F32, tag="mmq")
                    for ct in range(CT):
                        nc.tensor.matmul(
                            ps, wqk[:, ct, ft * 128:(ft + 1) * 128],
                            hkvT[ct][:, chh * 512:(chh + 1) * 512],
                            start=(ct == 0), stop=(ct == CT - 1))
                    nc.vector.tensor_copy(
                        out=qT[ft][:, chh * 512:(chh + 1) * 512], in_=ps)

            # ============ Phase A: LN1, transposes ============
            kv_stack = ExitStack()
            hkvT_pool = kv_stack.enter_context(
                tc.tile_pool(name="hkvT", bufs=1, side="right"))
            wqk_pool = kv_stack.enter_context(
                tc.tile_pool(name="wqkv_kq", bufs=1, side="right"))
            wv_stack = ExitStack()
            wqv_pool = wv_stack.enter_context(
                tc.tile_pool(name="wqkv_v", bufs=1, side="right"))
            hkvT.extend(hkvT_pool.tile([128, NKV], BF16, tag=f"hkvT{ct}",
                                       name=f"hkvT{ct}") for ct in range(CT))
            qkv_stack = ExitStack()
            emit.qkv_stack = qkv_stack
            vA_pool = qkv_stack.enter_context(
                tc.tile_pool(name="vA", bufs=1))
            vA.extend(vA_pool.tile([128, H, D + 1], BF16, tag=f"vA{nt}",
                                   name=f"vA{nt}") for nt in range(KT))
            wqk = wqk_pool.tile([128, CT, 2 * C], BF16, tag="wqkv_kq")
            wqv = wqv_pool.tile([128, CT, C], BF16, tag="wqkv_v")
            with (
                tc.tile_pool(name="ln1_work", bufs=2) as lw,
                tc.tile_pool(name="ln1_stat", bufs=6) as lstat,
            ):
                for g in range(KT // 4):  # groups of 4 token tiles
                    xts, xcs = [], []
                    for j in range(4):
                        nt = 4 * g + j
                        xt = lw.tile([128, C], BF16, tag=f"xt{j}",
                                     name=f"xt{j}")
                        nc.gpsimd.dma_start(
                            out=xt, in_=xb[nt * 128:(nt + 1) * 128, :])
                        st = lstat.tile([128, 3, 6], F32, tag="st")
                        xg = xt.rearrange("p (s d) -> p s d", s=3)
                        for s in range(3):
                            nc.vector.bn_stats(out=st[:, s], in_=xg[:, s])
                        mv = lstat.tile([128, 2], F32, tag="mv")
                        nc.vector.bn_aggr(out=mv, in_=st)
                        rstd = lstat.tile([128, 1], F32, tag="rstd")
                        nc.scalar.activation(out=rstd, in_=mv[:, 1:2],
                                             func=AF.Sqrt,
                                             bias=eps_t, scale=1.0)
                        nc.vector.reciprocal(out=rstd, in_=rstd)
                        nmr = lstat.tile([128, 1], F32, tag="nmr")
                        nc.vector.tensor_scalar(
                            out=nmr, in0=mv[:, 0:1], scalar1=-1.0,
                            scalar2=rstd, op0=ALU.mult, op1=ALU.mult)
                        xc = lw.tile([128, C], BF16, tag=f"xc{j}",
                                     name=f"xc{j}", bufs=1)
                        nc.scalar.activation(out=xc, in_=xt,
                                             func=AF.Identity,
                                             scale=rstd, bias=nmr)
                        xts.append(xt)
                        xcs.append(xc)
                    if g == 0:
                        # land in the DMA queue before QKV matmuls need them
                        nc.gpsimd.dma_start(out=wqv,
                            in_=wqkvT.rearrange("(ct p) f -> p ct f",
                                                p=128)[:, :, 2 * C:3 * C])
                        nc.gpsimd.dma_start(out=wqk,
                            in_=wqkvT.rearrange("(ct p) f -> p ct f",
                                                p=128)[:, :, 0:2 * C])
                    for ct in range(CT):
                        ps = sps.tile([128, 1024], BF16, tag="sp",
                                      name="ptr")
                        for j in range(4):
                            nc.tensor.transpose(
                                ps[:, j * 128:(j + 1) * 128],
                                xcs[j][:, ct * 128:(ct + 1) * 128],
                                ident_bf)
                        nc.scalar.activation(
                            out=hkvT[ct][:, g * 512:(g + 1) * 512],
                            in_=ps[:, 0:512], func=AF.Identity,
                            scale=g1_s[:, ct:ct + 1], bias=b1_s[:, ct:ct + 1])
                    if g < NO // 512:  # own tokens: raw x^T for residual
                        for ct in range(CT):
                            ps32 = sps.tile([128, 1024], BF16, tag="sp",
                                            name="ptr32")
                            for j in range(4):
                                nc.tensor.transpose(
                                    ps32[:, j * 128:(j + 1) * 128],
                                    xts[j][:, ct * 128:(ct + 1) * 128],
                                    ident_bf)
                            nc.vector.tensor_copy(
                                out=x2T[ct][:, g * 512:(g + 1) * 512],
                                in_=ps32[:, 0:512])
                    # ---- V for this group's token tiles (pipelined into
                    # phase A: fills PE slack under the LN1 DMA/stats chain
                    # so attention can start as soon as K/Q(ft0) is done) --
                    for j in range(4):
                        nt = 4 * g + j
                        psv = sps.tile([128, 1024], F32, tag="sp",
                                       name="psv")
                        for ct in range(CT):
                            hk = hkvT[ct][:, nt * 128:(nt + 1) * 128]
                            nc.tensor.matmul(psv[:, 0:512], hk,
                                             wqv[:, ct, 0:512],
                                             start=(ct == 0),
                                             stop=(ct == CT - 1))
                            nc.tensor.matmul(psv[:, 512:768], hk,
                                             wqv[:, ct, 512:C],
                                             start=(ct == 0),
                                             stop=(ct == CT - 1))
                        nc.vector.tensor_copy(
                            out=vA[nt][:, 0:8, 0:D],
                            in_=psv[:, 0:512].rearrange("p (h d) -> p h d",
                                                        d=D))
                        nc.vector.tensor_copy(
                            out=vA[nt][:, 8:12, 0:D],
                            in_=psv[:, 512:768].rearrange(
                                "p (h d) -> p h d", d=D))
                        nc.vector.memset(vA[nt][:, :, D:D + 1], 1.0)
                # proj weights: queue behind x/wq so LN1 isn't delayed
                nc.gpsimd.dma_start(out=wp,
                    in_=wprojT.rearrange("(ct p) f -> p ct f", p=128))

            wv_stack.close()  # wqv freed

            # ===== Phases C/D/E: K/Q + attention(ch0) + proj/LN2(ch0) =====
            qT_pool = qkv_stack.enter_context(
                tc.tile_pool(name="qT", bufs=1))
            kT_pool = qkv_stack.enter_context(
                tc.tile_pool(name="kT", bufs=1))
            qT.extend(qT_pool.tile([128, NO], BF16, tag=f"qT{ct}",
                                   name=f"qT{ct}") for ct in range(CT))
            kT.extend(kT_pool.tile([128, NKV], BF16, tag=f"kT{ct}",
                                   name=f"kT{ct}") for ct in range(CT))
            with tc.tile_pool(name="mmq_psum", bufs=2, space="PSUM") as mmq:
                emit.mmq = mmq
                kq_unit(0)
                for hp in range(CT):
                    if hp + 1 < CT:
                        kq_unit(hp + 1)
                    attn_hp(hp, 0)
                proj_ln2(0, mmq, "mmq")
                h2c_apply(0)
            kv_stack.close()  # hkvT + K/Q weights freed

            # ===== Phase F: attention(ch1) + MLP quarter (hb0, ch0) =====
            with (
                tc.tile_pool(name="wfc1f", bufs=1, side="right") as w1f_pool,
                tc.tile_pool(name="wfc2f", bufs=1, side="right") as w2f_pool,
                tc.tile_pool(name="f_psum", bufs=2, space="PSUM") as fps,
            ):
                fps._ktag = "f1"
                w1f = w1f_pool.tile([128, CT, HQ * 128], BF16, tag="w1f")
                nc.gpsimd.dma_start(out=w1f,
                    in_=wfc1T.rearrange("(ct p) f -> p ct f",
                                        p=128)[:, :, 0:HQ * 128])
                filler = [
                    [],
                    [make_fc1(fps, w1f, 0, 0, ht, defer_gelu=True)
                     for ht in range(3)],
                    [make_fc1(fps, w1f, 0, 0, ht, defer_gelu=True)
                     for ht in range(3, 6)],
                    [make_fc1(fps, w1f, 0, 0, ht, defer_gelu=True)
                     for ht in range(6, 9)],
                    [make_fc1(fps, w1f, 0, 0, ht, defer_gelu=True)
                     for ht in range(9, HQ)],
                    [],
                ]
                for hp in range(CT):
                    for u in filler[hp]:
                        u()
                    attn_hp(hp, 1)
                proj_ln2(1, fps, "f1")
        emit.qkv_stack.close()  # qT/kT/vA freed

        # ======= Phases G/H: LN2 apply ch1 + remaining MLP quarters =======
        with (
            tc.tile_pool(name="wfc1b", bufs=1, side="right") as w1b_pool,
            tc.tile_pool(name="wfc2b", bufs=1, side="right") as w2b_pool,
            tc.tile_pool(name="ft_psum", bufs=4, space="PSUM") as fpst,
        ):
            fpst._ktag = "ft"
            w1 = {}
            w2 = {}
            w2[0] = w2b_pool.tile([128, HQ, C], BF16, tag="w2h0r",
                                  name="w2h0r")
            nc.gpsimd.dma_start(out=w2[0],
                in_=wfc2T.rearrange("(ht p) f -> p ht f", p=128)[:, 0:HQ, :])
            w1[1] = w1b_pool.tile([128, CT, HQ * 128], BF16, tag="w1h1",
                                  name="w1h1")
            nc.gpsimd.dma_start(out=w1[1],
                in_=wfc1T.rearrange("(ct p) f -> p ct f",
                                    p=128)[:, :, HQ * 128:HID])
            w2[1] = w2b_pool.tile([128, HQ, C], BF16, tag="w2h1",
                                  name="w2h1")
            nc.gpsimd.dma_start(out=w2[1],
                in_=wfc2T.rearrange("(ht p) f -> p ht f", p=128)[:, HQ:HT, :])
            w1[0] = w1b_pool.tile([128, CT, HQ * 128], BF16, tag="w1h0r",
                                  name="w1h0r")
            nc.gpsimd.dma_start(out=w1[0],
                in_=wfc1T.rearrange("(ct p) f -> p ct f",
                                    p=128)[:, :, 0:HQ * 128])
            h2c_apply(1)
            # batched gelu for the F-phase fc1 quarter (one table load),
            # then its fc2 -> acc; then the remaining three quarters.
            gg = [w1b_pool.tile([128, 512], BF16, tag=f"gg{ht}",
                                name=f"gg{ht}") for ht in range(HQ)]
            for ht in range(HQ):
                nc.scalar.activation(out=gg[ht], in_=g_sb[ht], func=AF.Gelu,
                                     bias=f1b_s[:, ht:ht + 1], scale=1.0)
            for ft in range(CT):
                make_fc2(fpst, w2[0], 0, 0, ft, gs=gg)()
            for hb, ch in ((1, 0), (0, 1), (1, 1)):
                for ht in range(HQ):
                    make_fc1(fpst, w1[hb], hb, ch, ht)()
                for ft in range(CT):
                    make_fc2(fpst, w2[hb], hb, ch, ft)()


def kernel(**inputs):
    global _NC_CACHE, LAST_RESULTS
    import os
    ins = {k: np.ascontiguousarray(np.asarray(v, dtype=np.float32))
           for k, v in inputs.items()}
    if _NC_CACHE is None:
        _NC_CACHE = build_program()
    nc = _NC_CACHE

    import ml_dtypes
    bf = ml_dtypes.bfloat16
    shared = {
        "wqkvT": np.ascontiguousarray(ins["qkv_w"].T.astype(bf)),
        "wprojT": np.ascontiguousarray(ins["proj_w"].T.astype(bf)),
        "wfc1T": np.ascontiguousarray(ins["fc1_w"].T.astype(bf)),
        "wfc2T": np.ascontiguousarray(ins["fc2_w"].T.astype(bf)),
        "pb": ins["proj_b"], "f1b": ins["fc1_b"], "f2b": ins["fc2_b"],
        "g1": ins["ln1_g"], "b1": ins["ln1_b"],
        "g2": ins["ln2_g"], "b2": ins["ln2_b"],
    }
    in_maps = []
    for s in range(NCORES):
        b, half = s // 2, s % 2
        m = dict(shared)
        m["xb"] = np.ascontiguousarray(np.roll(ins["x"][b], -half * NO, axis=0))
        in_maps.append(m)

    trace = bool(int(os.environ.get("KBENCH_TRACE", "0")))
    LAST_RESULTS = run_bass_kernel_spmd(
        nc, in_maps, core_ids=list(range(NCORES)), trace=trace)
    out = np.empty((B, N, C), np.float32)
    for s in range(NCORES):
        b, half = s // 2, s % 2
        out[b, half * NO:(half + 1) * NO, :] = LAST_RESULTS.results[s]["outT"].T
    return out


# revision 28
# speedup vs baseline: 1.2437x; 1.0003x over previous
"""Trainium2 Bass kernel for a dense transformer block (B=4, N=2048, C=768,
H=12, D=64, HID=3072), sharded over 8 NeuronCores.

Sharding: token-split, no collectives. Core s handles batch b = s//2,
sequence half = s%2 (1024 tokens). Each core receives its batch element's
full 2048-token x (rolled so its own tokens are rows 0..1023), computes
K/V over all 2048 tokens (redundantly with its pair core), and produces
the output for its own 1024 tokens. Host gathers/transposes.

Layout: activations are kept feature-major ("X^T", [C, tokens]) so every
linear layer is a natural PE matmul (weights pre-transposed on host).
Attention computes S^T = K^T-tiles.T @ Q^T per head with softmax along
the partition (key) axis; the two heads of a feature-tile pair run
concurrently on the PE via tile_position row packing and share ONE
[128,1024] 2-bank PSUM tile so a single paired Exp activation covers
both. Denominators come from a ones-column appended to V; normalization
via GPSIMD partition-broadcast.

Pipelining: the kernel is emission-order software-pipelined so the
ScalarE exp stream (the attention bottleneck) always has PE work
running underneath it, keeping the PE HAM clock warm:
  D: attention(chunk0) interleaved with K/Q generation for the next
     head pair.
  F: attention(chunk1) interleaved with the first MLP quarter
     (fc1+fc2 of hidden-half 0, chunk 0).
  H: remaining three MLP quarters back-to-back (PE dense).
QKV/attention/proj and the MLP run in bf16 (fp32 PSUM accumulation).
"""

import numpy as np

import concourse.bass as bass
import concourse.mybir as mybir
import concourse.tile as tile
from concourse import bacc
from concourse.bass_utils import run_bass_kernel_spmd
from concourse.masks import make_identity

F32 = mybir.dt.float32
BF16 = mybir.dt.bfloat16
AF = mybir.ActivationFunctionType
ALU = mybir.AluOpType

B, N, C = 4, 2048, 768
H, D = 12, 64
HID = 3072
EPS = 1e-5
NCORES = 8
NO = 1024  # tokens owned per core
NKV = 2048  # key/value tokens per core
CT = C // 128  # 6 feature tiles
HT = HID // 128  # 24 hidden tiles
HQ = HT // 2  # 12 hidden tiles per half
KT = NKV // 128  # 16 kv token tiles
QCH = NO // 512  # 2 query chunks of 512
ISCALE = 1.0 / np.sqrt(D)

LAST_RESULTS = None
_NC_CACHE = None


def build_program(repeats=1):
    nc = bacc.Bacc(trn_type="TRN2", target_bir_lowering=False, num_devices=NCORES)

    xb = nc.dram_tensor("xb", [NKV, C], F32, kind="ExternalInput").ap()
    wqkvT = nc.dram_tensor("wqkvT", [C, 3 * C], BF16, kind="ExternalInput").ap()
    wprojT = nc.dram_tensor("wprojT", [C, C], BF16, kind="ExternalInput").ap()
    wfc1T = nc.dram_tensor("wfc1T", [C, HID], BF16, kind="ExternalInput").ap()
    wfc2T = nc.dram_tensor("wfc2T", [HID, C], BF16, kind="ExternalInput").ap()
    pb = nc.dram_tensor("pb", [C], F32, kind="ExternalInput").ap()
    f1b = nc.dram_tensor("f1b", [HID], F32, kind="ExternalInput").ap()
    f2b = nc.dram_tensor("f2b", [C], F32, kind="ExternalInput").ap()
    g1 = nc.dram_tensor("g1", [C], F32, kind="ExternalInput").ap()
    b1 = nc.dram_tensor("b1", [C], F32, kind="ExternalInput").ap()
    g2 = nc.dram_tensor("g2", [C], F32, kind="ExternalInput").ap()
    b2 = nc.dram_tensor("b2", [C], F32, kind="ExternalInput").ap()
    outT = nc.dram_tensor("outT", [C, NO], F32, kind="ExternalOutput").ap()

    with tile.TileContext(nc) as tc:
        for _ in range(repeats):
            emit(nc, tc, xb, wqkvT, wprojT, wfc1T, wfc2T, pb, f1b, f2b,
                 g1, b1, g2, b2, outT)
    nc.compile()
    return nc


def emit(nc, tc, xb, wqkvT, wprojT, wfc1T, wfc2T, pb, f1b, f2b,
         g1, b1, g2, b2, outT):
    dma = nc.sync.dma_start
    from contextlib import ExitStack

    with ExitStack() as top:
        lpool = lambda name, bufs: top.enter_context(
            tc.tile_pool(name=name, bufs=bufs))
        rpool = lambda name, bufs: top.enter_context(
            tc.tile_pool(name=name, bufs=bufs, side="right"))
        # ---- left side: constants + attention-phase working set ----
        consts = lpool("consts", 1)
        x2T_pool = lpool("x2T", 1)
        oT_pool = lpool("oT", 1)
        p_sb = lpool("p_sb", 3)
        asm = lpool("attn_sm", 2)
        wsb = lpool("work_sb", 2)
        wp_pool = lpool("wproj", 1)
        # ---- right side: MLP-persistent pools (live into phase H) ----
        g_pool = rpool("g_sb", 1)
        h2_pool = rpool("h2c", 1)
        acc_pool = rpool("acc_sb", 1)
        osb = rpool("out_sb", 2)
        bc_pool = rpool("bc_sb", 1)
        l2s = rpool("ln2_sm", 1)

        # ---- constants ----
        ident = consts.tile([128, 128], F32, tag="ident")
        make_identity(nc, ident)
        ident_bf = consts.tile([128, 128], BF16, tag="ident_bf")
        make_identity(nc, ident_bf)
        ones_f32 = consts.tile([128, 1], F32, tag="ones_f32")
        nc.vector.memset(ones_f32, 1.0)
        ones_cb = consts.tile([128, 1], BF16, tag="ones_cb")
        nc.scalar.activation(out=ones_cb, in_=ones_f32, func=AF.Copy)
        eps_t = consts.tile([128, 1], F32, tag="eps")
        nc.vector.memset(eps_t, EPS)
        # per-feature vectors as [128, CT] (col ct = features ct*128..)
        g1_s = consts.tile([128, CT], F32, tag="g1")
        dma(out=g1_s, in_=g1.rearrange("(ct p) -> p ct", p=128))
        b1_s = consts.tile([128, CT], F32, tag="b1")
        dma(out=b1_s, in_=b1.rearrange("(ct p) -> p ct", p=128))
        g2_s = consts.tile([128, CT], F32, tag="g2")
        dma(out=g2_s, in_=g2.rearrange("(ct p) -> p ct", p=128))
        b2_s = consts.tile([128, CT], F32, tag="b2")
        dma(out=b2_s, in_=b2.rearrange("(ct p) -> p ct", p=128))
        pb_s = consts.tile([128, CT], F32, tag="pb")
        dma(out=pb_s, in_=pb.rearrange("(ct p) -> p ct", p=128))
        f2b_s = consts.tile([128, CT], F32, tag="f2b")
        dma(out=f2b_s, in_=f2b.rearrange("(ct p) -> p ct", p=128))
        f1b_s = consts.tile([128, HT], F32, tag="f1b")
        dma(out=f1b_s, in_=f1b.rearrange("(ht p) -> p ht", p=128))

        x2T = [x2T_pool.tile([128, NO], BF16, tag=f"x2T{ct}", name=f"x2T{ct}")
               for ct in range(CT)]
        oT = [oT_pool.tile([128, 512], BF16, tag=f"oT{ct}", name=f"oT{ct}")
              for ct in range(CT)]
        wp = wp_pool.tile([128, CT, C], BF16, tag="wproj")
        g_sb = [g_pool.tile([128, 512], BF16, tag=f"g{ht}", name=f"g{ht}")
                for ht in range(HQ)]
        h2c = [[h2_pool.tile([128, 512], BF16, tag=f"h2_{ct}_{ch}",
                             name=f"h2_{ct}_{ch}") for ct in range(CT)]
               for ch in range(QCH)]
        acc = [[acc_pool.tile([128, 512], BF16, tag=f"acc{ft}_{ch}",
                              name=f"acc{ft}_{ch}") for ch in range(QCH)]
               for ft in range(CT)]
        ln2v = [None, None]
        qs = (slice(0, 64), slice(64, 128))
        qT, kT, vA, hkvT = [], [], [], []

        def h2c_apply(ch):
            """LN2 apply for chunk ch: h2c = (x2T*rstd + nmr)*g2 + b2 (DVE)."""
            cs = slice(ch * 512, (ch + 1) * 512)
            rstd, nmr = ln2v[ch]
            bc_r = bc_pool.tile([128, 512], F32, tag="bc_r")
            nc.gpsimd.partition_broadcast(bc_r, rstd, channels=128)
            bc_m = bc_pool.tile([128, 512], F32, tag="bc_m")
            nc.gpsimd.partition_broadcast(bc_m, nmr, channels=128)
            for ct in range(CT):
                t = osb.tile([128, 512], F32, tag="h2tmp")
                nc.vector.tensor_mul(t, x2T[ct][:, cs], bc_r)
                t2 = osb.tile([128, 512], F32, tag="h2tmp")
                nc.vector.tensor_add(t2, t, bc_m)
                nc.vector.tensor_scalar(
                    out=h2c[ch][ct], in0=t2,
                    scalar1=g2_s[:, ct:ct + 1], op0=ALU.mult,
                    scalar2=b2_s[:, ct:ct + 1], op1=ALU.add)

        def make_fc1(pool, w1t, hb, ch, ht, defer_gelu=False):
            def f():
                ps = pool.tile([128, 512], F32, tag=pool._ktag)
                for ct in range(CT):
                    nc.tensor.matmul(
                        ps, w1t[:, ct, ht * 128:(ht + 1) * 128], h2c[ch][ct],
                        start=(ct == 0), stop=(ct == CT - 1))
                hti = hb * HQ + ht
                if defer_gelu:
                    # raw pre-activation out; gelu runs batched in phase H
                    # so the exp ACT table isn't thrashed mid-attention.
                    nc.vector.tensor_copy(out=g_sb[ht], in_=ps)
                else:
                    nc.scalar.activation(out=g_sb[ht], in_=ps, func=AF.Gelu,
                                         bias=f1b_s[:, hti:hti + 1],
                                         scale=1.0)
            return f

        def make_fc2(pool, w2t, hb, ch, ft, gs=None):
            def f():
                g_src = gs if gs is not None else g_sb
                cs = slice(ch * 512, (ch + 1) * 512)
                ps = pool.tile([128, 512], F32, tag=pool._ktag)
                for ht in range(HQ):
                    nc.tensor.matmul(
                        ps, w2t[:, ht, ft * 128:(ft + 1) * 128], g_src[ht],
                        start=(ht == 0), stop=(ht == HQ - 1))
                if hb == 0:
                    nc.vector.scalar_tensor_tensor(
                        out=acc[ft][ch], in0=ps,
                        scalar=f2b_s[:, ft:ft + 1], in1=x2T[ft][:, cs],
                        op0=ALU.add, op1=ALU.add)
                else:
                    ot = osb.tile([128, 512], F32, tag="ot")
                    nc.vector.tensor_add(ot, ps, acc[ft][ch])
                    dma(out=outT[ft * 128:(ft + 1) * 128, cs], in_=ot)
            return f

        def proj_ln2(ch, pool, tag):
            """proj + residual into x2T (in place), then LN2 stats."""
            cs = slice(ch * 512, (ch + 1) * 512)
            for ft in range(CT):
                ps = pool.tile([128, 512], F32, tag=tag, name="ppj")
                for ct in range(CT):
                    nc.tensor.matmul(
                        ps, wp[:, ct, ft * 128:(ft + 1) * 128], oT[ct],
                        start=(ct == 0), stop=(ct == CT - 1))
                nc.vector.scalar_tensor_tensor(
                    out=x2T[ft][:, cs], in0=ps, scalar=pb_s[:, ft:ft + 1],
                    in1=x2T[ft][:, cs], op0=ALU.add, op1=ALU.add)
            psum = pool.tile([1, 512], F32, tag=tag, name="psum")
            pssq = pool.tile([1, 512], F32, tag=tag, name="pssq")
            for ct in range(CT):
                sq = wsb.tile([128, 512], BF16, tag="sq", bufs=1)
                nc.vector.tensor_mul(sq, x2T[ct][:, cs], x2T[ct][:, cs])
                nc.tensor.matmul(psum, ones_cb, x2T[ct][:, cs],
                                 start=(ct == 0), stop=(ct == CT - 1),
                                 skip_group_check=True)
                nc.tensor.matmul(pssq, ones_cb, sq,
                                 start=(ct == 0), stop=(ct == CT - 1),
                                 skip_group_check=True)
            mu = l2s.tile([1, 512], F32, tag="mu")
            nc.scalar.mul(mu, psum, 1.0 / C)
            msq = l2s.tile([1, 512], F32, tag="msq")
            nc.scalar.mul(msq, pssq, 1.0 / C)
            mu2 = l2s.tile([1, 512], F32, tag="tmpa")
            nc.vector.tensor_mul(mu2, mu, mu)
            # var in place over msq; rstd = exp(-0.5*ln(var+eps)) keeps the
            # ACT table in the natural_log_exp set (no Sqrt-set thrash
            # between the attention exp streams).
            nc.vector.tensor_sub(msq, msq, mu2)
            nc.scalar.activation(out=mu2, in_=msq, func=AF.Ln,
                                 bias=eps_t[0:1], scale=1.0)
            rstd = l2s.tile([1, 512], F32, tag=f"rstd2_{ch}", name="rstd2")
            nc.scalar.activation(out=rstd, in_=mu2, func=AF.Exp,
                                 scale=-0.5)
            nmr = l2s.tile([1, 512], F32, tag=f"nmr2_{ch}", name="nmr2")
            nc.vector.scalar_tensor_tensor(
                out=nmr, in0=mu, scalar=-1.0, in1=rstd,
                op0=ALU.mult, op1=ALU.mult)
            ln2v[ch] = (rstd, nmr)

        with (
            tc.tile_pool(name="sp_psum", bufs=2, space="PSUM") as sps,
            tc.tile_pool(name="o_psum", bufs=2, space="PSUM") as ops,
        ):
            def attn_hp(hp, ch):
                """Attention for head pair hp on query chunk ch."""
                qch = slice(ch * 512, (ch + 1) * 512)

                def s_pair(nt):
                    ps = sps.tile([128, 1024], F32, tag="sp", name="spS")
                    for i in range(2):
                        nc.tensor.matmul(
                            ps[:, i * 512:(i + 1) * 512],
                            kT[hp][qs[i], nt * 128:(nt + 1) * 128],
                            qT[hp][qs[i], qch],
                            start=True, stop=True,
                            tile_position=(64 * i, 0))
                    return ps

                po = [ops.tile([D + 1, 512], F32, tag="po", name="po")
                      for _ in range(2)]
                ps_cur = s_pair(0)
                for nt in range(KT):
                    ps_next = s_pair(nt + 1) if nt < KT - 1 else None
                    pt = p_sb.tile([128, 1024], BF16, tag="pt")
                    nc.scalar.activation(out=pt, in_=ps_cur,
                                         func=AF.Exp, scale=ISCALE)
                    for i in range(2):
                        nc.tensor.matmul(
                            po[i], vA[nt][:, 2 * hp + i, :],
                            pt[:, i * 512:(i + 1) * 512],
                            start=(nt == 0), stop=(nt == KT - 1),
                            skip_group_check=True)
                    ps_cur = ps_next
                for i in range(2):
                    # 1/den = exp(-ln(den)): Ln/Exp share the attention
                    # exp's ACT table set, so this slots into the exp
                    # stream with no table reload and no DVE reciprocal.
                    lden = asm.tile([1, 512], F32, tag="lden")
                    nc.scalar.activation(out=lden, in_=po[i][D:D + 1, :],
                                         func=AF.Ln)
                    rec = asm.tile([1, 512], BF16, tag="rec")
                    nc.scalar.activation(out=rec, in_=lden, func=AF.Exp,
                                         scale=-1.0)
                    vb = asm.tile([D, 512], BF16, tag="vb")
                    nc.gpsimd.partition_broadcast(vb, rec, channels=D)
                    nc.vector.tensor_mul(
                        oT[hp][qs[i], :], po[i][0:D, :], vb)

            def kq_unit(ft):
                for chh in range(NKV // 512):
                    ps = emit.mmq.tile([128, 512], F32, tag="mmq")
                    for ct in range(CT):
                        nc.tensor.matmul(
                            ps, wqk[:, ct, C + ft * 128:C + (ft + 1) * 128],
                            hkvT[ct][:, chh * 512:(chh + 1) * 512],
                            start=(ct == 0), stop=(ct == CT - 1))
                    nc.vector.tensor_copy(
                        out=kT[ft][:, chh * 512:(chh + 1) * 512], in_=ps)
                for chh in range(QCH):
                    ps = emit.mmq.tile([128, 512], F32, tag="mmq")
                    for ct in range(CT):
                        nc.tensor.matmul(
                            ps, wqk[:, ct, ft * 128:(ft + 1) * 128],
                            hkvT[ct][:, chh * 512:(chh + 1) * 512],
                            start=(ct == 0), stop=(ct == CT - 1))
                    nc.vector.tensor_copy(
                        out=qT[ft][:, chh * 512:(chh + 1) * 512], in_=ps)

            # ============ Phase A: LN1, transposes ============
            kv_stack = ExitStack()
            hkvT_pool = kv_stack.enter_context(
                tc.tile_pool(name="hkvT", bufs=1, side="right"))
            wqk_pool = kv_stack.enter_context(
                tc.tile_pool(name="wqkv_kq", bufs=1, side="right"))
            wv_stack = ExitStack()
            wqv_pool = wv_stack.enter_context(
                tc.tile_pool(name="wqkv_v", bufs=1, side="right"))
            hkvT.extend(hkvT_pool.tile([128, NKV], BF16, tag=f"hkvT{ct}",
                                       name=f"hkvT{ct}") for ct in range(CT))
            qkv_stack = ExitStack()
            emit.qkv_stack = qkv_stack
            vA_pool = qkv_stack.enter_context(
                tc.tile_pool(name="vA", bufs=1))
            vA.extend(vA_pool.tile([128, H, D + 1], BF16, tag=f"vA{nt}",
                                   name=f"vA{nt}") for nt in range(KT))
            wqk = wqk_pool.tile([128, CT, 2 * C], BF16, tag="wqkv_kq")
            wqv = wqv_pool.tile([128, CT, C], BF16, tag="wqkv_v")
            with (
                tc.tile_pool(name="ln1_work", bufs=2) as lw,
                tc.tile_pool(name="ln1_stat", bufs=6) as lstat,
            ):
                for g in range(KT // 4):  # groups of 4 token tiles
                    xts, xcs = [], []
                    for j in range(4):
                        nt = 4 * g + j
                        xt = lw.tile([128, C], BF16, tag=f"xt{j}",
                                     name=f"xt{j}")
                        nc.gpsimd.dma_start(
                            out=xt, in_=xb[nt * 128:(nt + 1) * 128, :])
                        st = lstat.tile([128, 3, 6], F32, tag="st")
                        xg = xt.rearrange("p (s d) -> p s d", s=3)
                        for s in range(3):
                            nc.vector.bn_stats(out=st[:, s], in_=xg[:, s])
                        mv = lstat.tile([128, 2], F32, tag="mv")
                        nc.vector.bn_aggr(out=mv, in_=st)
                        rstd = lstat.tile([128, 1], F32, tag="rstd")
                        nc.scalar.activation(out=rstd, in_=mv[:, 1:2],
                                             func=AF.Sqrt,
                                             bias=eps_t, scale=1.0)
                        nc.vector.reciprocal(out=rstd, in_=rstd)
                        nmr = lstat.tile([128, 1], F32, tag="nmr")
                        nc.vector.tensor_scalar(
                            out=nmr, in0=mv[:, 0:1], scalar1=-1.0,
                            scalar2=rstd, op0=ALU.mult, op1=ALU.mult)
                        xc = lw.tile([128, C], BF16, tag=f"xc{j}",
                                     name=f"xc{j}", bufs=1)
                        nc.scalar.activation(out=xc, in_=xt,
                                             func=AF.Identity,
                                             scale=rstd, bias=nmr)
                        xts.append(xt)
                        xcs.append(xc)
                    if g == 0:
                        # land in the DMA queue before QKV matmuls need them
                        nc.gpsimd.dma_start(out=wqv,
                            in_=wqkvT.rearrange("(ct p) f -> p ct f",
                                                p=128)[:, :, 2 * C:3 * C])
                        nc.gpsimd.dma_start(out=wqk,
                            in_=wqkvT.rearrange("(ct p) f -> p ct f",
                                                p=128)[:, :, 0:2 * C])
                    for ct in range(CT):
                        ps = sps.tile([128, 1024], BF16, tag="sp",
                                      name="ptr")
                        for j in range(4):
                            nc.tensor.transpose(
                                ps[:, j * 128:(j + 1) * 128],
                                xcs[j][:, ct * 128:(ct + 1) * 128],
                                ident_bf)
                        nc.scalar.activation(
                            out=hkvT[ct][:, g * 512:(g + 1) * 512],
                            in_=ps[:, 0:512], func=AF.Identity,
                            scale=g1_s[:, ct:ct + 1], bias=b1_s[:, ct:ct + 1])
                    if g < NO // 512:  # own tokens: raw x^T for residual
                        for ct in range(CT):
                            ps32 = sps.tile([128, 1024], BF16, tag="sp",
                                            name="ptr32")
                            for j in range(4):
                                nc.tensor.transpose(
                                    ps32[:, j * 128:(j + 1) * 128],
                                    xts[j][:, ct * 128:(ct + 1) * 128],
                                    ident_bf)
                            nc.vector.tensor_copy(
                                out=x2T[ct][:, g * 512:(g + 1) * 512],
                                in_=ps32[:, 0:512])
                    # ---- V for this group's token tiles (pipelined into
                    # phase A: fills PE slack under the LN1 DMA/stats chain
                    # so attention can start as soon as K/Q(ft0) is done) --
                    for j in range(4):
                        nt = 4 * g + j
                        psv = sps.tile([128, 1024], F32, tag="sp",
                                       name="psv")
                        for ct in range(CT):
                            hk = hkvT[ct][:, nt * 128:(nt + 1) * 128]
                            nc.tensor.matmul(psv[:, 0:512], hk,
                                             wqv[:, ct, 0:512],
                                             start=(ct == 0),
                                             stop=(ct == CT - 1))
                            nc.tensor.matmul(psv[:, 512:768], hk,
                                             wqv[:, ct, 512:C],
                                             start=(ct == 0),
                                             stop=(ct == CT - 1))
                        nc.vector.tensor_copy(
                            out=vA[nt][:, 0:8, 0:D],
                            in_=psv[:, 0:512].rearrange("p (h d) -> p h d",
                                                        d=D))
                        nc.vector.tensor_copy(
                            out=vA[nt][:, 8:12, 0:D],
                            in_=psv[:, 512:768].rearrange(
                                "p (h d) -> p h d", d=D))
                        nc.vector.memset(vA[nt][:, :, D:D + 1], 1.0)
                # proj weights: queue behind x/wq so LN1 isn't delayed
                nc.gpsimd.dma_start(out=wp,
                    in_=wprojT.rearrange("(ct p) f -> p ct f", p=128))

            wv_stack.close()  # wqv freed

            # ===== Phases C/D/E: K/Q + attention(ch0) + proj/LN2(ch0) =====
            qT_pool = qkv_stack.enter_context(
                tc.tile_pool(name="qT", bufs=1))
            kT_pool = qkv_stack.enter_context(
                tc.tile_pool(name="kT", bufs=1))
            qT.extend(qT_pool.tile([128, NO], BF16, tag=f"qT{ct}",
                                   name=f"qT{ct}") for ct in range(CT))
            kT.extend(kT_pool.tile([128, NKV], BF16, tag=f"kT{ct}",
                                   name=f"kT{ct}") for ct in range(CT))
            with tc.tile_pool(name="mmq_psum", bufs=2, space="PSUM") as mmq:
                emit.mmq = mmq
                kq_unit(0)
                for hp in range(CT):
                    if hp + 1 < CT:
                        kq_unit(hp + 1)
                    attn_hp(hp, 0)
                proj_ln2(0, mmq, "mmq")
                h2c_apply(0)
            kv_stack.close()  # hkvT + K/Q weights freed

            # ===== Phase F: attention(ch1) + MLP quarter (hb0, ch0) =====
            with (
                tc.tile_pool(name="wfc1f", bufs=1, side="right") as w1f_pool,
                tc.tile_pool(name="wfc2f", bufs=1, side="right") as w2f_pool,
                tc.tile_pool(name="f_psum", bufs=2, space="PSUM") as fps,
            ):
                fps._ktag = "f1"
                w1f = w1f_pool.tile([128, CT, HQ * 128], BF16, tag="w1f")
                nc.gpsimd.dma_start(out=w1f,
                    in_=wfc1T.rearrange("(ct p) f -> p ct f",
                                        p=128)[:, :, 0:HQ * 128])
                filler = [
                    [],
                    [make_fc1(fps, w1f, 0, 0, ht, defer_gelu=True)
                     for ht in range(3)],
                    [make_fc1(fps, w1f, 0, 0, ht, defer_gelu=True)
                     for ht in range(3, 6)],
                    [make_fc1(fps, w1f, 0, 0, ht, defer_gelu=True)
                     for ht in range(6, 9)],
                    [make_fc1(fps, w1f, 0, 0, ht, defer_gelu=True)
                     for ht in range(9, HQ)],
                    [],
                ]
                for hp in range(CT):
                    for u in filler[hp]:
                        u()
                    attn_hp(hp, 1)
                proj_ln2(1, fps, "f1")
        emit.qkv_stack.close()  # qT/kT/vA freed

        # ======= Phases G/H: LN2 apply ch1 + remaining MLP quarters =======
        with (
            tc.tile_pool(name="wfc1b", bufs=1, side="right") as w1b_pool,
            tc.tile_pool(name="wfc2b", bufs=1, side="right") as w2b_pool,
            tc.tile_pool(name="ft_psum", bufs=6, space="PSUM") as fpst,
        ):
            fpst._ktag = "ft"
            w1 = {}
            w2 = {}
            w2[0] = w2b_pool.tile([128, HQ, C], BF16, tag="w2h0r",
                                  name="w2h0r")
            nc.gpsimd.dma_start(out=w2[0],
                in_=wfc2T.rearrange("(ht p) f -> p ht f", p=128)[:, 0:HQ, :])
            w1[1] = w1b_pool.tile([128, CT, HQ * 128], BF16, tag="w1h1",
                                  name="w1h1")
            nc.gpsimd.dma_start(out=w1[1],
                in_=wfc1T.rearrange("(ct p) f -> p ct f",
                                    p=128)[:, :, HQ * 128:HID])
            w2[1] = w2b_pool.tile([128, HQ, C], BF16, tag="w2h1",
                                  name="w2h1")
            nc.gpsimd.dma_start(out=w2[1],
                in_=wfc2T.rearrange("(ht p) f -> p ht f", p=128)[:, HQ:HT, :])
            w1[0] = w1b_pool.tile([128, CT, HQ * 128], BF16, tag="w1h0r",
                                  name="w1h0r")
            nc.gpsimd.dma_start(out=w1[0],
                in_=wfc1T.rearrange("(ct p) f -> p ct f",
                                    p=128)[:, :, 0:HQ * 128])
            # batched gelu for the F-phase fc1 quarter (one table load);
            # emitted before h2c_apply so ScalarE isn't queued behind the
            # DVE LN2-apply chain. Then its fc2 -> acc; then the
            # remaining three quarters.
            gg = [w1b_pool.tile([128, 512], BF16, tag=f"gg{ht}",
                                name=f"gg{ht}") for ht in range(HQ)]
            for ht in range(HQ):
                nc.scalar.activation(out=gg[ht], in_=g_sb[ht], func=AF.Gelu,
                                     bias=f1b_s[:, ht:ht + 1], scale=1.0)
            h2c_apply(1)
            for ft in range(CT):
                make_fc2(fpst, w2[0], 0, 0, ft, gs=gg)()
            for hb, ch in ((1, 0), (0, 1), (1, 1)):
                for ht in range(HQ):
                    make_fc1(fpst, w1[hb], hb, ch, ht)()
                for ft in range(CT):
                    make_fc2(fpst, w2[hb], hb, ch, ft)()


def kernel(**inputs):
    global _NC_CACHE, LAST_RESULTS
    import os
    ins = {k: np.ascontiguousarray(np.asarray(v, dtype=np.float32))
           for k, v in inputs.items()}
    if _NC_CACHE is None:
        _NC_CACHE = build_program()
    nc = _NC_CACHE

    import ml_dtypes
    bf = ml_dtypes.bfloat16
    shared = {
        "wqkvT": np.ascontiguousarray(ins["qkv_w"].T.astype(bf)),
        "wprojT": np.ascontiguousarray(ins["proj_w"].T.astype(bf)),
        "wfc1T": np.ascontiguousarray(ins["fc1_w"].T.astype(bf)),
        "wfc2T": np.ascontiguousarray(ins["fc2_w"].T.astype(bf)),
        "pb": ins["proj_b"], "f1b": ins["fc1_b"], "f2b": ins["fc2_b"],
        "g1": ins["ln1_g"], "b1": ins["ln1_b"],
        "g2": ins["ln2_g"], "b2": ins["ln2_b"],
    }
    in_maps = []
    for s in range(NCORES):
        b, half = s // 2, s % 2
        m = dict(shared)
        m["xb"] = np.ascontiguousarray(np.roll(ins["x"][b], -half * NO, axis=0))
        in_maps.append(m)

    trace = bool(int(os.environ.get("KBENCH_TRACE", "0")))
    LAST_RESULTS = run_bass_kernel_spmd(
        nc, in_maps, core_ids=list(range(NCORES)), trace=trace)
    out = np.empty((B, N, C), np.float32)
    for s in range(NCORES):
        b, half = s // 2, s % 2
        out[b, half * NO:(half + 1) * NO, :] = LAST_RESULTS.results[s]["outT"].T
    return out
